# revision 19
# baseline (speedup 1.0000x reference)
"""Trainium2 Bass kernel for a CLIP encoder layer (B=32, S=257, E=1024, H=16, I=4096).

Strategy: data-parallel over batch across 8 NeuronCores (4 batch elements per
core), no collectives.  Per-core compute is done feature-major ([E, tokens])
so projection matmuls need no on-device transposes:

  - LayerNorm: column stats via PE ones-matmuls (f32r), normalization applied
    with two DVE passes; LN scale/bias are folded into the projection weights
    on the host.
  - Q/K/O/fc1/fc2: weight-stationary matmuls (lhsT = W^T packed on host,
    bf16), fp32 PSUM accumulation, N=257 (one batch element) moving slices.
  - V: activation-stationary -> token-major [tok, H, 65] with a ones column,
    so the softmax denominators fall out of the ctx matmul for free.
  - Attention: scores computed transposed (scores^T[j, i]) so softmax reduces
    over the partition dim via the ctx matmul; exp on ACT straight from PSUM;
    two heads (D=64) packed per PE pass at partition bases 0/64.
"""

import numpy as np
import ml_dtypes

B, S, E, H, D, II = 32, 257, 1024, 16, 64, 4096
N_CORES = 8
B_LOC = B // N_CORES          # 4
NT = B_LOC * S                # 1028
KC = E // 128                 # 8
MC_E = E // 128               # 8
MC_I = II // 128              # 32
EPS = 1e-5

# token slices within NT, used for f32r matmuls (N>=256 except the 4-tail)
LN_SLICES = [(0, 512), (512, 1024), (1024, NT)]
# j-chunks of one batch element's 257 keys
JC = [(0, 128), (128, 128), (256, 1)]

TRACE = False
LAST_EXEC_NS = None

_cache = {}


def _build(with_mask: bool):
    import concourse.tile as tile
    from concourse import bacc, mybir
    from contextlib import ExitStack

    F32 = mybir.dt.float32
    BF16 = mybir.dt.bfloat16
    F32R = mybir.dt.float32r
    AF = mybir.ActivationFunctionType
    ALU = mybir.AluOpType

    nc = bacc.Bacc("TRN2", target_bir_lowering=False, debug=False,
                   enable_asserts=False, num_devices=N_CORES)

    xT_d = nc.dram_tensor("xT", [E, NT], F32, kind="ExternalInput")
    xTb_d = nc.dram_tensor("xTb", [E, NT], BF16, kind="ExternalInput")
    qw_d = nc.dram_tensor("qw", [MC_E, 128, KC, 128], BF16, kind="ExternalInput")
    kw_d = nc.dram_tensor("kw", [MC_E, 128, KC, 128], BF16, kind="ExternalInput")
    vw_d = nc.dram_tensor("vw", [KC, 128, E], BF16, kind="ExternalInput")
    ow_d = nc.dram_tensor("ow", [MC_E, 128, KC, 128], BF16, kind="ExternalInput")
    f1w_d = nc.dram_tensor("f1w", [MC_I, 128, KC, 128], BF16, kind="ExternalInput")
    f2w_d = nc.dram_tensor("f2w", [MC_E, 128, MC_I, 128], BF16, kind="ExternalInput")
    qb_d = nc.dram_tensor("qb", [128, MC_E], F32, kind="ExternalInput")
    kb_d = nc.dram_tensor("kb", [128, MC_E], F32, kind="ExternalInput")
    vb_d = nc.dram_tensor("vb", [1, E], F32, kind="ExternalInput")
    ob_d = nc.dram_tensor("ob", [128, MC_E], F32, kind="ExternalInput")
    f1b_d = nc.dram_tensor("f1b", [128, MC_I], F32, kind="ExternalInput")
    f2b_d = nc.dram_tensor("f2b", [128, MC_E], F32, kind="ExternalInput")
    mskT_d = None
    if with_mask:
        mskT_d = nc.dram_tensor("mskT", [B_LOC, S, S], F32, kind="ExternalInput")
    outT_d = nc.dram_tensor("outT", [E, NT], F32, kind="ExternalOutput")

    with tile.TileContext(nc) as tc, ExitStack() as top:
        consts = top.enter_context(tc.tile_pool(name="consts", bufs=1))

        # ---- constants / biases -------------------------------------
        ones_col = consts.tile([128, 1], BF16)
        nc.vector.memset(ones_col[:], 1.0)
        ones_row = consts.tile([1, 128], BF16)
        nc.vector.memset(ones_row[:], 1.0)
        eps_t = consts.tile([1, 1], F32)
        nc.vector.memset(eps_t[:], EPS)
        qb_sb = consts.tile([128, MC_E], F32)
        nc.sync.dma_start(out=qb_sb[:], in_=qb_d[:])
        kb_sb = consts.tile([128, MC_E], F32)
        nc.sync.dma_start(out=kb_sb[:], in_=kb_d[:])
        ob_sb = consts.tile([128, MC_E], F32)
        nc.sync.dma_start(out=ob_sb[:], in_=ob_d[:])
        f2b_sb = consts.tile([128, MC_E], F32)
        nc.sync.dma_start(out=f2b_sb[:], in_=f2b_d[:])
        f1b_sb = consts.tile([128, MC_I], F32)
        nc.sync.dma_start(out=f1b_sb[:], in_=f1b_d[:])
        vb_sb = consts.tile([128, E], F32)
        nc.sync.dma_start(out=vb_sb[:], in_=vb_d[0:1, :].to_broadcast((128, E)))

        def emit_ln(src, out_pool, sfx, src_bf=None):
            """Column LayerNorm (over the partition/feature dim) of 8
            [128, NT] f32 tiles -> 8 [128, NT] bf16 normalized tiles
            (scale/bias NOT applied; folded into weights host-side).
            If src_bf is given, stats are computed from those bf16 tiles."""
            outs = []
            with ExitStack() as ph:
                lntmp = ph.enter_context(
                    tc.tile_pool(name=f"lntmp{sfx}", bufs=3))
                rows = ph.enter_context(tc.tile_pool(name=f"rows{sfx}", bufs=6))
                with tc.tile_pool(name=f"pstat{sfx}", bufs=2,
                                  space="PSUM") as pstat:
                    ps_sum = pstat.tile([1, NT], F32, name="ps_sum", tag="stat")
                    ps_sq = pstat.tile([1, NT], F32, name="ps_sq", tag="stat")
                    for k in range(KC):
                        if src_bf is not None:
                            xb = src_bf[k]
                        else:
                            xb = lntmp.tile([128, NT], BF16, name="xb",
                                            tag="xb")
                            nc.vector.tensor_copy(out=xb[:], in_=src[k][:])
                        sq = lntmp.tile([128, NT], BF16, name="sq", tag="sq")
                        nc.scalar.activation(out=sq[:], in_=xb[:],
                                             func=AF.Square)
                        for (a, b) in LN_SLICES:
                            nc.tensor.matmul(ps_sum[0:1, a:b],
                                             ones_col[:],
                                             xb[:, a:b],
                                             start=(k == 0), stop=(k == KC - 1))
                            nc.tensor.matmul(ps_sq[0:1, a:b],
                                             ones_col[:],
                                             sq[:, a:b],
                                             start=(k == 0), stop=(k == KC - 1))
                    # musq = (sum * -1/E)^2 ; muneg_b = sum * -1/E (bf16)
                    musq = rows.tile([1, NT], F32, name="musq", tag="row")
                    nc.scalar.activation(out=musq[0:1, :], in_=ps_sum[0:1, :],
                                         func=AF.Square, scale=-1.0 / E)
                    muneg_b = rows.tile([1, NT], BF16, name="muneg_b",
                                        tag="row")
                    nc.scalar.mul(out=muneg_b[0:1, :], in_=ps_sum[0:1, :],
                                  mul=-1.0 / E)
                    var = rows.tile([1, NT], F32, name="var", tag="row")
                    nc.vector.scalar_tensor_tensor(
                        out=var[0:1, :], in0=ps_sq[0:1, :], scalar=1.0 / E,
                        in1=musq[0:1, :], op0=ALU.mult, op1=ALU.subtract)
                sd = rows.tile([1, NT], F32, name="sd", tag="row")
                nc.scalar.activation(out=sd[0:1, :], in_=var[0:1, :],
                                     func=AF.Sqrt, bias=eps_t[0:1, 0:1])
                rstd = rows.tile([1, NT], F32, name="rstd", tag="row")
                nc.vector.reciprocal_approx_fast(out=rstd[0:1, :],
                                                 in_=sd[0:1, :])
                rstd_b = rows.tile([1, NT], BF16, name="rstd_b", tag="row")
                nc.vector.tensor_copy(out=rstd_b[0:1, :], in_=rstd[0:1, :])
                with tc.tile_pool(name=f"pbc{sfx}", bufs=2,
                                  space="PSUM") as pbc:
                    psA = pbc.tile([128, NT], F32, name="psA", tag="bc")
                    psB = pbc.tile([128, NT], F32, name="psB", tag="bc")
                    for (a, b) in LN_SLICES:
                        nc.tensor.matmul(psA[:, a:b],
                                         ones_row[0:1, :],
                                         rstd_b[0:1, a:b],
                                         start=True, stop=True)
                        nc.tensor.matmul(psB[:, a:b],
                                         ones_row[0:1, :],
                                         muneg_b[0:1, a:b],
                                         start=True, stop=True)
                    # x_ln = (x - mu) * rstd
                    for k in range(KC):
                        tmp = lntmp.tile([128, NT], F32, name="tmp", tag="ap")
                        nc.vector.tensor_add(out=tmp[:], in0=src[k][:],
                                             in1=psB[:])
                        o = out_pool.tile([128, NT], BF16, name="lno",
                                          tag="lno")
                        nc.vector.tensor_mul(out=o[:], in0=tmp[:], in1=psA[:])
                        outs.append(o)
            return outs

        with tc.tile_pool(name="xt", bufs=KC) as xt_p:
            with tc.tile_pool(name="ctxT", bufs=MC_E) as ctx_p:
                ctxT = [ctx_p.tile([128, NT], BF16, tag="ctxT", name="ctxT")
                        for _ in range(MC_E)]

                # ============= LN1 / V / QK+attention ====================
                with tc.tile_pool(name="xln1", bufs=KC) as xln1_p:
                    with tc.tile_pool(name="xtb", bufs=KC) as xtb_p:
                        xtb = []
                        for k in range(KC):
                            tb = xtb_p.tile([128, NT], BF16, name="xtb",
                                            tag="xtb")
                            nc.sync.dma_start(
                                out=tb[:],
                                in_=xTb_d[k * 128:(k + 1) * 128, :])
                            xtb.append(tb)
                        xt = []
                        for k in range(KC):
                            t = xt_p.tile([128, NT], F32, name="xt", tag="xt")
                            nc.sync.dma_start(
                                out=t[:],
                                in_=xT_d[k * 128:(k + 1) * 128, :])
                            xt.append(t)
                        xln1 = emit_ln(xt, xln1_p, "1", src_bf=xtb)

                    with tc.tile_pool(name="vpool", bufs=3 * B_LOC) as v_p:
                        v_tiles = {}
                        with ExitStack() as ph:
                            vw_p = ph.enter_context(
                                tc.tile_pool(name="vw", bufs=1))
                            ppv = ph.enter_context(
                                tc.tile_pool(name="ppv", bufs=3, space="PSUM"))
                            vw_sb = vw_p.tile([128, KC, E], BF16)
                            for k in range(KC):
                                nc.sync.dma_start(out=vw_sb[:, k, :],
                                                  in_=vw_d[k, :, :])
                            for b in range(B_LOC):
                                for jc, (j0, jcs) in enumerate(JC):
                                    ps = ppv.tile([128, 2, 512], F32,
                                                  name="vps", tag="vps")
                                    for n in range(2):
                                        for k in range(KC):
                                            nc.tensor.matmul(
                                                ps[0:jcs, n, :],
                                                xln1[k][:, b * S + j0:
                                                        b * S + j0 + jcs],
                                                vw_sb[:, k,
                                                      n * 512:(n + 1) * 512],
                                                start=(k == 0),
                                                stop=(k == KC - 1))
                                    # [tok, H, 128]: cols 0:64 ones, cols
                                    # 64:128 V -> ctx matmul replicates the
                                    # softmax sums across partitions 0:64
                                    # (base 0: custom-DVE recip needs it).
                                    vt = v_p.tile([128, H, 128], BF16,
                                                  name="vt", tag="vt")
                                    nc.vector.tensor_add(out=vt[0:jcs, :, 64:128],
                                                         in0=ps[0:jcs, :, :],
                                                         in1=vb_sb[0:jcs, :])
                                    nc.vector.memset(vt[:, :, 0:64], 1.0)
                                    v_tiles[(b, jc)] = vt

                        with ExitStack() as ph:
                            qt_p = ph.enter_context(
                                tc.tile_pool(name="qt", bufs=2))
                            kt_p = ph.enter_context(
                                tc.tile_pool(name="kt", bufs=2))
                            wqk_p = ph.enter_context(
                                tc.tile_pool(name="wqk", bufs=3))
                            e_p = ph.enter_context(
                                tc.tile_pool(name="ep", bufs=6))
                            rs_p = ph.enter_context(
                                tc.tile_pool(name="rsp", bufs=3))
                            if with_mask:
                                msk_p = ph.enter_context(
                                    tc.tile_pool(name="mskp", bufs=3 * B_LOC))
                            psp = ph.enter_context(
                                tc.tile_pool(name="psp", bufs=2, space="PSUM"))
                            pcp = ph.enter_context(
                                tc.tile_pool(name="pcp", bufs=2, space="PSUM"))
                            if with_mask:
                                msk = {}
                                for b in range(B_LOC):
                                    for jc, (j0, jcs) in enumerate(JC):
                                        mt = msk_p.tile([128, S], F32,
                                                        name="mt", tag="mt")
                                        nc.sync.dma_start(
                                            out=mt[0:jcs, :],
                                            in_=mskT_d[b, j0:j0 + jcs, :])
                                        msk[(b, jc)] = mt

                            for ec in range(MC_E):
                                # Q / K chunk ec
                                qkt = []
                                for (w_d, b_sb, opool) in (
                                        (qw_d, qb_sb, qt_p),
                                        (kw_d, kb_sb, kt_p)):
                                    wt = wqk_p.tile([128, KC, 128], BF16,
                                                    name="wqk", tag="wqk")
                                    nc.sync.dma_start(out=wt[:],
                                                      in_=w_d[ec, :, :, :])
                                    ot = opool.tile([128, NT], BF16,
                                                    name="qk", tag="qk")
                                    for half in range(2):
                                        ps = pcp.tile([128, 2, 512], F32,
                                                      name="pqk", tag="cp")
                                        for bb in range(2):
                                            bsl = (half * 2 + bb) * S
                                            for k in range(KC):
                                                nc.tensor.matmul(
                                                    ps[:, bb, 0:S],
                                                    wt[:, k, :],
                                                    xln1[k][:, bsl:bsl + S],
                                                    start=(k == 0),
                                                    stop=(k == KC - 1))
                                        nc.vector.tensor_scalar_add(
                                            out=ot[:, half * 2 * S:
                                                   (half + 1) * 2 * S],
                                            in0=ps[:, :, 0:S],
                                            scalar1=b_sb[:, ec:ec + 1])
                                    qkt.append(ot)
                                qte, kte = qkt

                                # attention for heads (2*ec, 2*ec+1)
                                for b in range(B_LOC):
                                    ets = []
                                    for jc, (j0, jcs) in enumerate(JC):
                                        sp = psp.tile([128, 2, 512], F32,
                                                      name="sp", tag="sp")
                                        for hi in range(2):
                                            p0 = hi * 64
                                            nc.tensor.matmul(
                                                sp[0:jcs, hi, 0:S],
                                                kte[p0:p0 + 64,
                                                    b * S + j0:
                                                    b * S + j0 + jcs],
                                                qte[p0:p0 + 64,
                                                    b * S:(b + 1) * S],
                                                start=True, stop=True)
                                        if with_mask:
                                            for hi in range(2):
                                                nc.vector.tensor_add(
                                                    out=sp[0:jcs, hi, 0:S],
                                                    in0=sp[0:jcs, hi, 0:S],
                                                    in1=msk[(b, jc)][0:jcs, :])
                                        et = e_p.tile([128, 2, S], BF16,
                                                      name="et", tag="et")
                                        nc.scalar.activation(
                                            out=et[0:jcs, :, :],
                                            in_=sp[0:jcs, :, 0:S],
                                            func=AF.Exp)
                                        ets.append(et)
                                    cp = pcp.tile([128, 2, 512], F32,
                                                  name="cp", tag="cp")
                                    for hi in range(2):
                                        h = 2 * ec + hi
                                        for jc, (j0, jcs) in enumerate(JC):
                                            nc.tensor.matmul(
                                                cp[0:128, hi, 0:S],
                                                v_tiles[(b, jc)][0:jcs, h, :],
                                                ets[jc][0:jcs, hi, :],
                                                start=(jc == 0),
                                                stop=(jc == 2))
                                    rst = rs_p.tile([64, 2, S], F32,
                                                    name="rst", tag="rst")
                                    nc.vector.reciprocal_approx_fast(
                                        out=rst[0:64, :, :],
                                        in_=cp[0:64, :, 0:S])
                                    for hi in range(2):
                                        nc.vector.tensor_mul(
                                            out=ctxT[ec][hi * 64:hi * 64 + 64,
                                                         b * S:(b + 1) * S],
                                            in0=cp[64:128, hi, 0:S],
                                            in1=rst[0:64, hi, :])

                # xln1 / v closed here; right-side long-lived pools
                ht_p = top.enter_context(
                    tc.tile_pool(name="ht", bufs=KC, side="right"))
                f1o_p = top.enter_context(
                    tc.tile_pool(name="f1o", bufs=MC_I, side="right"))

                # ============= out projection + residual =================
                ht = []
                with ExitStack() as ph:
                    wo_p = ph.enter_context(tc.tile_pool(name="wo", bufs=3))
                    ppo = ph.enter_context(
                        tc.tile_pool(name="ppo", bufs=2, space="PSUM"))
                    for m in range(MC_E):
                        wt = wo_p.tile([128, KC, 128], BF16, name="wo",
                                       tag="wo")
                        nc.sync.dma_start(out=wt[:], in_=ow_d[m, :, :, :])
                        ps = ppo.tile([128, B_LOC, 512], F32, name="po",
                                      tag="po")
                        for b in range(B_LOC):
                            for k in range(KC):
                                nc.tensor.matmul(
                                    ps[:, b, 0:S], wt[:, k, :],
                                    ctxT[k][:, b * S:(b + 1) * S],
                                    start=(k == 0), stop=(k == KC - 1))
                        o = ht_p.tile([128, NT], F32, name="ht", tag="ht")
                        nc.vector.scalar_tensor_tensor(
                            out=o[:], in0=ps[:, :, 0:S],
                            scalar=ob_sb[:, m:m + 1], in1=xt[m][:],
                            op0=ALU.add, op1=ALU.add)
                        ht.append(o)
            # ctxT closed
        # xt closed

        # ================= LN2 + MLP =====================================
        with tc.tile_pool(name="xln2", bufs=KC) as xln2_p:
            xln2 = emit_ln(ht, xln2_p, "2")
            f1o = []
            with ExitStack() as ph:
                wf1_p = ph.enter_context(tc.tile_pool(name="wf1", bufs=3))
                ppf1 = ph.enter_context(
                    tc.tile_pool(name="ppf1", bufs=2, space="PSUM"))
                for m in range(MC_I):
                    wt = wf1_p.tile([128, KC, 128], BF16, name="wf1",
                                    tag="wf1")
                    nc.sync.dma_start(out=wt[:], in_=f1w_d[m, :, :, :])
                    ps = ppf1.tile([128, B_LOC, 512], F32, name="pf1",
                                   tag="pf1")
                    for b in range(B_LOC):
                        for k in range(KC):
                            nc.tensor.matmul(
                                ps[:, b, 0:S], wt[:, k, :],
                                xln2[k][:, b * S:(b + 1) * S],
                                start=(k == 0), stop=(k == KC - 1))
                    o = f1o_p.tile([128, NT], BF16, name="f1o", tag="f1o")
                    nc.scalar.activation(out=o[:], in_=ps[:, :, 0:S],
                                         func=AF.Gelu_apprx_tanh,
                                         bias=f1b_sb[:, m:m + 1])
                    f1o.append(o)

        with ExitStack() as ph:
            wf2_p = ph.enter_context(tc.tile_pool(name="wf2", bufs=2))
            ppf2 = ph.enter_context(
                tc.tile_pool(name="ppf2", bufs=2, space="PSUM"))
            out_p = ph.enter_context(tc.tile_pool(name="outp", bufs=3))
            for m in range(MC_E):
                wt = wf2_p.tile([128, MC_I, 128], BF16, name="wf2", tag="wf2")
                nc.sync.dma_start(out=wt[:], in_=f2w_d[m, :, :, :])
                ps = ppf2.tile([128, B_LOC, 512], F32, name="pf2", tag="pf2")
                for b in range(B_LOC):
                    for k in range(MC_I):
                        nc.tensor.matmul(
                            ps[:, b, 0:S], wt[:, k, :],
                            f1o[k][:, b * S:(b + 1) * S],
                            start=(k == 0), stop=(k == MC_I - 1))
                o = out_p.tile([128, NT], F32, name="oo", tag="oo")
                nc.vector.scalar_tensor_tensor(
                    out=o[:], in0=ps[:, :, 0:S], scalar=f2b_sb[:, m:m + 1],
                    in1=ht[m][:], op0=ALU.add, op1=ALU.add)
                nc.sync.dma_start(out=outT_d[m * 128:(m + 1) * 128, :],
                                  in_=o[:])

    nc.compile()
    return nc


def _pack_lhsT(W):
    """W [M, K] (out, in) -> [M/128, 128, K/128, 128] bf16 with
    [m, p, k, j] = W[m*128+j, k*128+p] (lhsT tiles, partition = K)."""
    W = np.asarray(W, np.float32)
    M, K = W.shape
    A = W.reshape(M // 128, 128, K // 128, 128)
    return np.ascontiguousarray(A.transpose(0, 3, 2, 1)).astype(ml_dtypes.bfloat16)


def _pack_pbias(b):
    """b [M] -> [128, M/128] f32 per-partition bias columns."""
    return np.ascontiguousarray(np.asarray(b, np.float32).reshape(-1, 128).T)


def kernel(hidden_states, attention_mask, causal_attention_mask,
           ln1_w, ln1_b, q_w, q_b, k_w, k_b, v_w, v_b, o_w, o_b,
           ln2_w, ln2_b, fc1_w, fc1_b, fc2_w, fc2_b):
    global LAST_EXEC_NS
    from concourse.bass_utils import run_bass_kernel_spmd

    hs = np.asarray(hidden_states, np.float32)
    msk = (np.asarray(attention_mask, np.float32)
           + np.asarray(causal_attention_mask, np.float32))
    with_mask = bool(np.any(msk))

    ln1_w = np.asarray(ln1_w, np.float32); ln1_b = np.asarray(ln1_b, np.float32)
    ln2_w = np.asarray(ln2_w, np.float32); ln2_b = np.asarray(ln2_b, np.float32)
    q_w = np.asarray(q_w, np.float32); q_b = np.asarray(q_b, np.float32)
    k_w = np.asarray(k_w, np.float32); k_b = np.asarray(k_b, np.float32)
    v_w = np.asarray(v_w, np.float32); v_b = np.asarray(v_b, np.float32)
    o_w = np.asarray(o_w, np.float32); o_b = np.asarray(o_b, np.float32)
    fc1_w = np.asarray(fc1_w, np.float32); fc1_b = np.asarray(fc1_b, np.float32)
    fc2_w = np.asarray(fc2_w, np.float32); fc2_b = np.asarray(fc2_b, np.float32)

    scale = D ** -0.5
    # fold LN1 scale/bias into Q/K/V, and the softmax scale into Q
    qw_eff = (q_w * ln1_w[None, :]) * scale
    qb_eff = (q_b + q_w @ ln1_b) * scale
    kw_eff = k_w * ln1_w[None, :]
    kb_eff = k_b + k_w @ ln1_b
    vw_eff = v_w * ln1_w[None, :]
    vb_eff = v_b + v_w @ ln1_b
    # fold LN2 into fc1
    f1w_eff = fc1_w * ln2_w[None, :]
    f1b_eff = fc1_b + fc1_w @ ln2_b

    base = {
        "qw": _pack_lhsT(qw_eff),
        "kw": _pack_lhsT(kw_eff),
        "vw": np.ascontiguousarray(
            vw_eff.T.reshape(KC, 128, E)).astype(ml_dtypes.bfloat16),
        "ow": _pack_lhsT(o_w),
        "f1w": _pack_lhsT(f1w_eff),
        "f2w": _pack_lhsT(fc2_w),
        "qb": _pack_pbias(qb_eff),
        "kb": _pack_pbias(kb_eff),
        "vb": np.ascontiguousarray(vb_eff[None, :].astype(np.float32)),
        "ob": _pack_pbias(o_b),
        "f1b": _pack_pbias(f1b_eff),
        "f2b": _pack_pbias(fc2_b),
    }

    key = with_mask
    if key not in _cache:
        _cache[key] = _build(with_mask)
    nc = _cache[key]

    in_maps = []
    for c in range(N_CORES):
        x = hs[c * B_LOC:(c + 1) * B_LOC].reshape(NT, E).T
        m = dict(base)
        m["xT"] = np.ascontiguousarray(x)
        m["xTb"] = np.ascontiguousarray(x).astype(ml_dtypes.bfloat16)
        if with_mask:
            m["mskT"] = np.ascontiguousarray(
                msk[c * B_LOC:(c + 1) * B_LOC, 0].transpose(0, 2, 1))
        in_maps.append(m)

    res = run_bass_kernel_spmd(nc, in_maps, core_ids=list(range(N_CORES)),
                               trace=TRACE)
    LAST_EXEC_NS = res.exec_time_ns

    outs = []
    for c in range(N_CORES):
        oT = res.results[c]["outT"]          # [E, NT] f32
        outs.append(np.ascontiguousarray(oT.T).reshape(B_LOC, S, E))
    return np.concatenate(outs, axis=0)


# revision 21
# speedup vs baseline: 1.0377x; 1.0377x over previous
"""Trainium2 Bass kernel for a CLIP encoder layer (B=32, S=257, E=1024, H=16, I=4096).

Strategy: data-parallel over batch across 8 NeuronCores (4 batch elements per
core), no collectives.  Per-core compute is done feature-major ([E, tokens])
so projection matmuls need no on-device transposes:

  - LayerNorm: column stats via PE ones-matmuls (f32r), normalization applied
    with two DVE passes; LN scale/bias are folded into the projection weights
    on the host.
  - Q/K/O/fc1/fc2: weight-stationary matmuls (lhsT = W^T packed on host,
    bf16), fp32 PSUM accumulation, N=257 (one batch element) moving slices.
  - V: activation-stationary -> token-major [tok, H, 65] with a ones column,
    so the softmax denominators fall out of the ctx matmul for free.
  - Attention: scores computed transposed (scores^T[j, i]) so softmax reduces
    over the partition dim via the ctx matmul; exp on ACT straight from PSUM;
    two heads (D=64) packed per PE pass at partition bases 0/64.
"""

import numpy as np
import ml_dtypes

B, S, E, H, D, II = 32, 257, 1024, 16, 64, 4096
N_CORES = 8
B_LOC = B // N_CORES          # 4
NT = B_LOC * S                # 1028
KC = E // 128                 # 8
MC_E = E // 128               # 8
MC_I = II // 128              # 32
EPS = 1e-5

# token slices within NT, used for f32r matmuls (N>=256 except the 4-tail)
LN_SLICES = [(0, 512), (512, 1024), (1024, NT)]
# j-chunks of one batch element's 257 keys
JC = [(0, 128), (128, 128), (256, 1)]

TRACE = False
LAST_EXEC_NS = None

_cache = {}


def _build(with_mask: bool):
    import concourse.tile as tile
    from concourse import bacc, mybir
    from contextlib import ExitStack

    F32 = mybir.dt.float32
    BF16 = mybir.dt.bfloat16
    AF = mybir.ActivationFunctionType
    ALU = mybir.AluOpType

    nc = bacc.Bacc("TRN2", target_bir_lowering=False, debug=False,
                   enable_asserts=False, num_devices=N_CORES)

    xT_d = nc.dram_tensor("xT", [E, NT], F32, kind="ExternalInput")
    xTb_d = nc.dram_tensor("xTb", [E, NT], BF16, kind="ExternalInput")
    qw_d = nc.dram_tensor("qw", [MC_E, 128, KC, 128], BF16, kind="ExternalInput")
    kw_d = nc.dram_tensor("kw", [MC_E, 128, KC, 128], BF16, kind="ExternalInput")
    vw_d = nc.dram_tensor("vw", [KC, 128, E], BF16, kind="ExternalInput")
    ow_d = nc.dram_tensor("ow", [MC_E, 128, KC, 128], BF16, kind="ExternalInput")
    f1w_d = nc.dram_tensor("f1w", [MC_I, 128, KC, 128], BF16, kind="ExternalInput")
    f2w_d = nc.dram_tensor("f2w", [MC_E, 128, MC_I, 128], BF16, kind="ExternalInput")
    qb_d = nc.dram_tensor("qb", [128, MC_E], F32, kind="ExternalInput")
    kb_d = nc.dram_tensor("kb", [128, MC_E], F32, kind="ExternalInput")
    vb_d = nc.dram_tensor("vb", [1, E], F32, kind="ExternalInput")
    ob_d = nc.dram_tensor("ob", [128, MC_E], F32, kind="ExternalInput")
    f1b_d = nc.dram_tensor("f1b", [128, MC_I], F32, kind="ExternalInput")
    f2b_d = nc.dram_tensor("f2b", [128, MC_E], F32, kind="ExternalInput")
    mskT_d = None
    if with_mask:
        mskT_d = nc.dram_tensor("mskT", [B_LOC, S, S], F32, kind="ExternalInput")
    outT_d = nc.dram_tensor("outT", [E, NT], F32, kind="ExternalOutput")

    with tile.TileContext(nc) as tc, ExitStack() as top:
        consts = top.enter_context(tc.tile_pool(name="consts", bufs=1))

        ones_col = consts.tile([128, 1], BF16)
        nc.vector.memset(ones_col[:], 1.0)
        ones_row = consts.tile([1, 128], BF16)
        nc.vector.memset(ones_row[:], 1.0)
        eps_t = consts.tile([1, 1], F32)
        nc.vector.memset(eps_t[:], EPS)
        qb_sb = consts.tile([128, MC_E], F32)
        nc.sync.dma_start(out=qb_sb[:], in_=qb_d[:])
        kb_sb = consts.tile([128, MC_E], F32)
        nc.sync.dma_start(out=kb_sb[:], in_=kb_d[:])
        ob_sb = consts.tile([128, MC_E], F32)
        nc.sync.dma_start(out=ob_sb[:], in_=ob_d[:])
        f2b_sb = consts.tile([128, MC_E], F32)
        nc.sync.dma_start(out=f2b_sb[:], in_=f2b_d[:])
        f1b_sb = consts.tile([128, MC_I], F32)
        nc.sync.dma_start(out=f1b_sb[:], in_=f1b_d[:])
        vb_sb = consts.tile([128, E], F32)
        nc.sync.dma_start(out=vb_sb[:], in_=vb_d[0:1, :].to_broadcast((128, E)))

        def emit_ln(ph, src_ap, srcbf_ap, out_pool, sfx):
            """Per-batch column LayerNorm over the feature (partition) dim.
            src_ap(k, b) -> [128, S] f32 AP; srcbf_ap(k, b) -> [128, S] bf16
            AP.  Returns {(k, b): [128, S] bf16 tile} of (x - mu) * rstd
            (LN scale/bias folded into downstream weights host-side)."""
            lntmp = ph.enter_context(tc.tile_pool(name=f"lntmp{sfx}", bufs=3))
            sqp = ph.enter_context(tc.tile_pool(name=f"sqp{sfx}", bufs=3))
            rows = ph.enter_context(tc.tile_pool(name=f"rows{sfx}", bufs=8))
            pstat = ph.enter_context(
                tc.tile_pool(name=f"pstat{sfx}", bufs=2, space="PSUM"))
            pbc = ph.enter_context(
                tc.tile_pool(name=f"pbc{sfx}", bufs=2, space="PSUM"))
            outs = {}
            for b in range(B_LOC):
                ps_sum = pstat.tile([1, 512], F32, name="ps_sum", tag="stat")
                ps_sq = pstat.tile([1, 512], F32, name="ps_sq", tag="stat")
                for k in range(KC):
                    xb = srcbf_ap(k, b)
                    sq = sqp.tile([128, S], BF16, name="sq", tag="sq")
                    nc.scalar.activation(out=sq[:], in_=xb, func=AF.Square)
                    nc.tensor.matmul(ps_sum[0:1, 0:S], ones_col[:], xb,
                                     start=(k == 0), stop=(k == KC - 1))
                    nc.tensor.matmul(ps_sq[0:1, 0:S], ones_col[:], sq[:],
                                     start=(k == 0), stop=(k == KC - 1))
                musq = rows.tile([1, S], F32, name="musq", tag="row")
                nc.scalar.activation(out=musq[0:1, :], in_=ps_sum[0:1, 0:S],
                                     func=AF.Square, scale=-1.0 / E)
                muneg_b = rows.tile([1, S], BF16, name="muneg_b", tag="row")
                nc.scalar.mul(out=muneg_b[0:1, :], in_=ps_sum[0:1, 0:S],
                              mul=-1.0 / E)
                var = rows.tile([1, S], F32, name="var", tag="row")
                nc.vector.scalar_tensor_tensor(
                    out=var[0:1, :], in0=ps_sq[0:1, 0:S], scalar=1.0 / E,
                    in1=musq[0:1, :], op0=ALU.mult, op1=ALU.subtract)
                sd = rows.tile([1, S], F32, name="sd", tag="row")
                nc.scalar.activation(out=sd[0:1, :], in_=var[0:1, :],
                                     func=AF.Sqrt, bias=eps_t[0:1, 0:1])
                rstd = rows.tile([1, S], F32, name="rstd", tag="row")
                nc.vector.reciprocal_approx_fast(out=rstd[0:1, :],
                                                 in_=sd[0:1, :])
                rstd_b = rows.tile([1, S], BF16, name="rstd_b", tag="row")
                nc.vector.tensor_copy(out=rstd_b[0:1, :], in_=rstd[0:1, :])
                psA = pbc.tile([128, 512], F32, name="psA", tag="bc")
                psB = pbc.tile([128, 512], F32, name="psB", tag="bc")
                nc.tensor.matmul(psA[:, 0:S], ones_row[0:1, :],
                                 rstd_b[0:1, :], start=True, stop=True)
                nc.tensor.matmul(psB[:, 0:S], ones_row[0:1, :],
                                 muneg_b[0:1, :], start=True, stop=True)
                for k in range(KC):
                    tmp = lntmp.tile([128, S], F32, name="tmp", tag="ap")
                    nc.vector.tensor_add(out=tmp[:], in0=src_ap(k, b),
                                         in1=psB[:, 0:S])
                    o = out_pool.tile([128, S], BF16, name="lno", tag="lno")
                    nc.vector.tensor_mul(out=o[:], in0=tmp[:], in1=psA[:, 0:S])
                    outs[(k, b)] = o
            return outs

        with tc.tile_pool(name="xt", bufs=KC * B_LOC) as xt_p:
            with tc.tile_pool(name="ctxT", bufs=MC_E) as ctx_p:
                ctxT = [ctx_p.tile([128, NT], BF16, tag="ctxT", name="ctxT")
                        for _ in range(MC_E)]

                # ============= LN1 / V / QK+attention ====================
                with tc.tile_pool(name="xln1", bufs=KC * B_LOC) as xln1_p:
                    with ExitStack() as ln1_ph:
                        xtb_p = ln1_ph.enter_context(
                            tc.tile_pool(name="xtb", bufs=KC * B_LOC))
                        xtb, xt = {}, {}
                        for b in range(B_LOC):
                            for k in range(KC):
                                tb = xtb_p.tile([128, S], BF16, name="xtb",
                                                tag="xtb")
                                nc.sync.dma_start(
                                    out=tb[:],
                                    in_=xTb_d[k * 128:(k + 1) * 128,
                                              b * S:(b + 1) * S])
                                xtb[(k, b)] = tb
                        for b in range(B_LOC):
                            for k in range(KC):
                                t = xt_p.tile([128, S], F32, name="xt",
                                              tag="xt")
                                nc.sync.dma_start(
                                    out=t[:],
                                    in_=xT_d[k * 128:(k + 1) * 128,
                                             b * S:(b + 1) * S])
                                xt[(k, b)] = t
                        xln1 = emit_ln(ln1_ph,
                                       lambda k, b: xt[(k, b)][:],
                                       lambda k, b: xtb[(k, b)][:],
                                       xln1_p, "1")

                        # ============= V projection ======================
                        v_p = ln1_ph.enter_context(
                            tc.tile_pool(name="vpool", bufs=3 * B_LOC))
                        v_tiles = {}
                        with ExitStack() as ph:
                            vw_p = ph.enter_context(
                                tc.tile_pool(name="vw", bufs=1))
                            ppv = ph.enter_context(
                                tc.tile_pool(name="ppv", bufs=2, space="PSUM"))
                            vw_sb = vw_p.tile([128, KC, E], BF16)
                            for k in range(KC):
                                nc.sync.dma_start(out=vw_sb[:, k, :],
                                                  in_=vw_d[k, :, :])
                            for b in range(B_LOC):
                                for jc, (j0, jcs) in enumerate(JC):
                                    ps = ppv.tile([128, 2, 512], F32,
                                                  name="vps", tag="vps")
                                    for n in range(2):
                                        for k in range(KC):
                                            nc.tensor.matmul(
                                                ps[0:jcs, n, :],
                                                xln1[(k, b)][:, j0:j0 + jcs],
                                                vw_sb[:, k,
                                                      n * 512:(n + 1) * 512],
                                                start=(k == 0),
                                                stop=(k == KC - 1))
                                    # [tok, H, 128]: cols 0:64 ones, cols
                                    # 64:128 V -> ctx matmul replicates the
                                    # softmax sums across partitions 0:64
                                    # (base 0: custom-DVE recip needs it).
                                    vt = v_p.tile([128, H, 128], BF16,
                                                  name="vt", tag="vt")
                                    nc.vector.tensor_add(
                                        out=vt[0:jcs, :, 64:128],
                                        in0=ps[0:jcs, :, :],
                                        in1=vb_sb[0:jcs, :])
                                    nc.vector.memset(vt[:, :, 0:64], 1.0)
                                    v_tiles[(b, jc)] = vt

                    # ========= Q/K + attention (per head-pair chunk) =====
                    with ExitStack() as ph:
                        qt_p = ph.enter_context(tc.tile_pool(name="qt", bufs=2))
                        kt_p = ph.enter_context(tc.tile_pool(name="kt", bufs=2))
                        wqk_p = ph.enter_context(
                            tc.tile_pool(name="wqk", bufs=3))
                        e_p = ph.enter_context(tc.tile_pool(name="ep", bufs=6))
                        rs_p = ph.enter_context(tc.tile_pool(name="rsp", bufs=3))
                        if with_mask:
                            msk_p = ph.enter_context(
                                tc.tile_pool(name="mskp", bufs=3 * B_LOC))
                        pp2 = ph.enter_context(
                            tc.tile_pool(name="pp2", bufs=1, space="PSUM"))
                        psp = ph.enter_context(
                            tc.tile_pool(name="psp", bufs=2, space="PSUM"))
                        pcp = ph.enter_context(
                            tc.tile_pool(name="pcp", bufs=1, space="PSUM"))
                        if with_mask:
                            msk = {}
                            for b in range(B_LOC):
                                for jc, (j0, jcs) in enumerate(JC):
                                    mt = msk_p.tile([128, S], F32, name="mt",
                                                    tag="mt")
                                    nc.sync.dma_start(
                                        out=mt[0:jcs, :],
                                        in_=mskT_d[b, j0:j0 + jcs, :])
                                    msk[(b, jc)] = mt

                        for ec in range(MC_E):
                            qkt = []
                            for (w_d, b_sb, opool) in (
                                    (qw_d, qb_sb, qt_p),
                                    (kw_d, kb_sb, kt_p)):
                                wt = wqk_p.tile([128, KC, 128], BF16,
                                                name="wqk", tag="wqk")
                                nc.sync.dma_start(out=wt[:],
                                                  in_=w_d[ec, :, :, :])
                                ot = opool.tile([128, NT], BF16,
                                                name="qk", tag="qk")
                                for half in range(2):
                                    ps = pp2.tile([128, 2, 512], F32,
                                                  name="pqk", tag="pqk")
                                    for bb in range(2):
                                        b = half * 2 + bb
                                        for k in range(KC):
                                            nc.tensor.matmul(
                                                ps[:, bb, 0:S],
                                                wt[:, k, :],
                                                xln1[(k, b)][:],
                                                start=(k == 0),
                                                stop=(k == KC - 1))
                                    nc.vector.tensor_scalar_add(
                                        out=ot[:, half * 2 * S:
                                               (half + 1) * 2 * S],
                                        in0=ps[:, :, 0:S],
                                        scalar1=b_sb[:, ec:ec + 1])
                                qkt.append(ot)
                            qte, kte = qkt

                            for b in range(B_LOC):
                                ets = []
                                for jc, (j0, jcs) in enumerate(JC):
                                    sp = psp.tile([128, 2, 512], F32,
                                                  name="sp", tag="sp")
                                    for hi in range(2):
                                        p0 = hi * 64
                                        nc.tensor.matmul(
                                            sp[0:jcs, hi, 0:S],
                                            kte[p0:p0 + 64,
                                                b * S + j0: b * S + j0 + jcs],
                                            qte[p0:p0 + 64,
                                                b * S:(b + 1) * S],
                                            start=True, stop=True)
                                    if with_mask:
                                        for hi in range(2):
                                            nc.vector.tensor_add(
                                                out=sp[0:jcs, hi, 0:S],
                                                in0=sp[0:jcs, hi, 0:S],
                                                in1=msk[(b, jc)][0:jcs, :])
                                    et = e_p.tile([128, 2, S], BF16,
                                                  name="et", tag="et")
                                    nc.scalar.activation(
                                        out=et[0:jcs, :, :],
                                        in_=sp[0:jcs, :, 0:S], func=AF.Exp)
                                    ets.append(et)
                                cp = pcp.tile([128, 2, 512], F32,
                                              name="cp", tag="cp")
                                for hi in range(2):
                                    h = 2 * ec + hi
                                    for jc, (j0, jcs) in enumerate(JC):
                                        nc.tensor.matmul(
                                            cp[0:128, hi, 0:S],
                                            v_tiles[(b, jc)][0:jcs, h, :],
                                            ets[jc][0:jcs, hi, :],
                                            start=(jc == 0), stop=(jc == 2))
                                rst = rs_p.tile([64, 2, S], F32,
                                                name="rst", tag="rst")
                                nc.vector.reciprocal_approx_fast(
                                    out=rst[0:64, :, :],
                                    in_=cp[0:64, :, 0:S])
                                for hi in range(2):
                                    nc.vector.tensor_mul(
                                        out=ctxT[ec][hi * 64:hi * 64 + 64,
                                                     b * S:(b + 1) * S],
                                        in0=cp[64:128, hi, 0:S],
                                        in1=rst[0:64, hi, :])

                # xln1 closed; right-side long-lived pools
                ht_p = top.enter_context(
                    tc.tile_pool(name="ht", bufs=KC, side="right"))
                f1o_p = top.enter_context(
                    tc.tile_pool(name="f1o", bufs=MC_I, side="right"))

                # ============= out projection + residual =================
                ht = []
                with ExitStack() as ph:
                    wo_p = ph.enter_context(tc.tile_pool(name="wo", bufs=3))
                    ppo = ph.enter_context(
                        tc.tile_pool(name="ppo", bufs=2, space="PSUM"))
                    for m in range(MC_E):
                        wt = wo_p.tile([128, KC, 128], BF16, name="wo",
                                       tag="wo")
                        nc.sync.dma_start(out=wt[:], in_=ow_d[m, :, :, :])
                        ps = ppo.tile([128, B_LOC, 512], F32, name="po",
                                      tag="po")
                        for b in range(B_LOC):
                            for k in range(KC):
                                nc.tensor.matmul(
                                    ps[:, b, 0:S], wt[:, k, :],
                                    ctxT[k][:, b * S:(b + 1) * S],
                                    start=(k == 0), stop=(k == KC - 1))
                        o = ht_p.tile([128, NT], F32, name="ht", tag="ht")
                        for b in range(B_LOC):
                            nc.vector.scalar_tensor_tensor(
                                out=o[:, b * S:(b + 1) * S],
                                in0=ps[:, b, 0:S],
                                scalar=ob_sb[:, m:m + 1],
                                in1=xt[(m, b)][:],
                                op0=ALU.add, op1=ALU.add)
                        ht.append(o)
            # ctxT closed
        # xt closed

        # ================= LN2 + MLP =====================================
        with tc.tile_pool(name="xln2", bufs=KC * B_LOC) as xln2_p:
            f1o = []
            with ExitStack() as ln2_ph:
                htb_p = ln2_ph.enter_context(tc.tile_pool(name="htb", bufs=KC))
                htb = []
                for k in range(KC):
                    hb = htb_p.tile([128, NT], BF16, name="htb", tag="htb")
                    nc.vector.tensor_copy(out=hb[:], in_=ht[k][:])
                    htb.append(hb)
                xln2 = emit_ln(
                    ln2_ph,
                    lambda k, b: ht[k][:, b * S:(b + 1) * S],
                    lambda k, b: htb[k][:, b * S:(b + 1) * S],
                    xln2_p, "2")
                wf1_p = ln2_ph.enter_context(tc.tile_pool(name="wf1", bufs=3))
                ppf1 = ln2_ph.enter_context(
                    tc.tile_pool(name="ppf1", bufs=2, space="PSUM"))
                for m in range(MC_I):
                    wt = wf1_p.tile([128, KC, 128], BF16, name="wf1",
                                    tag="wf1")
                    nc.sync.dma_start(out=wt[:], in_=f1w_d[m, :, :, :])
                    o = f1o_p.tile([128, NT], BF16, name="f1o", tag="f1o")
                    for half in range(2):
                        ps = ppf1.tile([128, 2, 512], F32, name="pf1",
                                       tag="pf1")
                        for bb in range(2):
                            b = half * 2 + bb
                            for k in range(KC):
                                nc.tensor.matmul(
                                    ps[:, bb, 0:S], wt[:, k, :],
                                    xln2[(k, b)][:],
                                    start=(k == 0), stop=(k == KC - 1))
                        nc.scalar.activation(
                            out=o[:, half * 2 * S:(half + 1) * 2 * S],
                            in_=ps[:, :, 0:S],
                            func=AF.Gelu_apprx_tanh,
                            bias=f1b_sb[:, m:m + 1])
                    f1o.append(o)

        with ExitStack() as ph:
            wf2_p = ph.enter_context(tc.tile_pool(name="wf2", bufs=2))
            ppf2 = ph.enter_context(
                tc.tile_pool(name="ppf2", bufs=2, space="PSUM"))
            out_p = ph.enter_context(tc.tile_pool(name="outp", bufs=3))
            for m in range(MC_E):
                wt = wf2_p.tile([128, MC_I, 128], BF16, name="wf2", tag="wf2")
                nc.sync.dma_start(out=wt[:], in_=f2w_d[m, :, :, :])
                ps = ppf2.tile([128, B_LOC, 512], F32, name="pf2", tag="pf2")
                for b in range(B_LOC):
                    for k in range(MC_I):
                        nc.tensor.matmul(
                            ps[:, b, 0:S], wt[:, k, :],
                            f1o[k][:, b * S:(b + 1) * S],
                            start=(k == 0), stop=(k == MC_I - 1))
                o = out_p.tile([128, NT], F32, name="oo", tag="oo")
                nc.vector.scalar_tensor_tensor(
                    out=o[:], in0=ps[:, :, 0:S], scalar=f2b_sb[:, m:m + 1],
                    in1=ht[m][:], op0=ALU.add, op1=ALU.add)
                nc.sync.dma_start(out=outT_d[m * 128:(m + 1) * 128, :],
                                  in_=o[:])

    nc.compile()
    return nc


def _pack_lhsT(W):
    """W [M, K] (out, in) -> [M/128, 128, K/128, 128] bf16 with
    [m, p, k, j] = W[m*128+j, k*128+p] (lhsT tiles, partition = K)."""
    W = np.asarray(W, np.float32)
    M, K = W.shape
    A = W.reshape(M // 128, 128, K // 128, 128)
    return np.ascontiguousarray(A.transpose(0, 3, 2, 1)).astype(ml_dtypes.bfloat16)


def _pack_pbias(b):
    """b [M] -> [128, M/128] f32 per-partition bias columns."""
    return np.ascontiguousarray(np.asarray(b, np.float32).reshape(-1, 128).T)


def kernel(hidden_states, attention_mask, causal_attention_mask,
           ln1_w, ln1_b, q_w, q_b, k_w, k_b, v_w, v_b, o_w, o_b,
           ln2_w, ln2_b, fc1_w, fc1_b, fc2_w, fc2_b):
    global LAST_EXEC_NS
    from concourse.bass_utils import run_bass_kernel_spmd

    hs = np.asarray(hidden_states, np.float32)
    msk = (np.asarray(attention_mask, np.float32)
           + np.asarray(causal_attention_mask, np.float32))
    with_mask = bool(np.any(msk))

    ln1_w = np.asarray(ln1_w, np.float32); ln1_b = np.asarray(ln1_b, np.float32)
    ln2_w = np.asarray(ln2_w, np.float32); ln2_b = np.asarray(ln2_b, np.float32)
    q_w = np.asarray(q_w, np.float32); q_b = np.asarray(q_b, np.float32)
    k_w = np.asarray(k_w, np.float32); k_b = np.asarray(k_b, np.float32)
    v_w = np.asarray(v_w, np.float32); v_b = np.asarray(v_b, np.float32)
    o_w = np.asarray(o_w, np.float32); o_b = np.asarray(o_b, np.float32)
    fc1_w = np.asarray(fc1_w, np.float32); fc1_b = np.asarray(fc1_b, np.float32)
    fc2_w = np.asarray(fc2_w, np.float32); fc2_b = np.asarray(fc2_b, np.float32)

    scale = D ** -0.5
    # fold LN1 scale/bias into Q/K/V, and the softmax scale into Q
    qw_eff = (q_w * ln1_w[None, :]) * scale
    qb_eff = (q_b + q_w @ ln1_b) * scale
    kw_eff = k_w * ln1_w[None, :]
    kb_eff = k_b + k_w @ ln1_b
    vw_eff = v_w * ln1_w[None, :]
    vb_eff = v_b + v_w @ ln1_b
    # fold LN2 into fc1
    f1w_eff = fc1_w * ln2_w[None, :]
    f1b_eff = fc1_b + fc1_w @ ln2_b

    base = {
        "qw": _pack_lhsT(qw_eff),
        "kw": _pack_lhsT(kw_eff),
        "vw": np.ascontiguousarray(
            vw_eff.T.reshape(KC, 128, E)).astype(ml_dtypes.bfloat16),
        "ow": _pack_lhsT(o_w),
        "f1w": _pack_lhsT(f1w_eff),
        "f2w": _pack_lhsT(fc2_w),
        "qb": _pack_pbias(qb_eff),
        "kb": _pack_pbias(kb_eff),
        "vb": np.ascontiguousarray(vb_eff[None, :].astype(np.float32)),
        "ob": _pack_pbias(o_b),
        "f1b": _pack_pbias(f1b_eff),
        "f2b": _pack_pbias(fc2_b),
    }

    key = with_mask
    if key not in _cache:
        _cache[key] = _build(with_mask)
    nc = _cache[key]

    in_maps = []
    for c in range(N_CORES):
        x = hs[c * B_LOC:(c + 1) * B_LOC].reshape(NT, E).T
        m = dict(base)
        m["xT"] = np.ascontiguousarray(x)
        m["xTb"] = np.ascontiguousarray(x).astype(ml_dtypes.bfloat16)
        if with_mask:
            m["mskT"] = np.ascontiguousarray(
                msk[c * B_LOC:(c + 1) * B_LOC, 0].transpose(0, 2, 1))
        in_maps.append(m)

    res = run_bass_kernel_spmd(nc, in_maps, core_ids=list(range(N_CORES)),
                               trace=TRACE)
    LAST_EXEC_NS = res.exec_time_ns

    outs = []
    for c in range(N_CORES):
        oT = res.results[c]["outT"]          # [E, NT] f32
        outs.append(np.ascontiguousarray(oT.T).reshape(B_LOC, S, E))
    return np.concatenate(outs, axis=0)


# revision 22
# speedup vs baseline: 1.0641x; 1.0254x over previous
"""Trainium2 Bass kernel for a CLIP encoder layer (B=32, S=257, E=1024, H=16, I=4096).

Strategy: data-parallel over batch across 8 NeuronCores (4 batch elements per
core), no collectives.  Per-core compute is done feature-major ([E, tokens])
so projection matmuls need no on-device transposes:

  - LayerNorm: column stats via PE ones-matmuls (f32r), normalization applied
    with two DVE passes; LN scale/bias are folded into the projection weights
    on the host.
  - Q/K/O/fc1/fc2: weight-stationary matmuls (lhsT = W^T packed on host,
    bf16), fp32 PSUM accumulation, N=257 (one batch element) moving slices.
  - V: activation-stationary -> token-major [tok, H, 65] with a ones column,
    so the softmax denominators fall out of the ctx matmul for free.
  - Attention: scores computed transposed (scores^T[j, i]) so softmax reduces
    over the partition dim via the ctx matmul; exp on ACT straight from PSUM;
    two heads (D=64) packed per PE pass at partition bases 0/64.
"""

import numpy as np
import ml_dtypes

B, S, E, H, D, II = 32, 257, 1024, 16, 64, 4096
N_CORES = 8
B_LOC = B // N_CORES          # 4
NT = B_LOC * S                # 1028
KC = E // 128                 # 8
MC_E = E // 128               # 8
MC_I = II // 128              # 32
EPS = 1e-5

# token slices within NT, used for f32r matmuls (N>=256 except the 4-tail)
LN_SLICES = [(0, 512), (512, 1024), (1024, NT)]
# j-chunks of one batch element's 257 keys
JC = [(0, 128), (128, 128), (256, 1)]

TRACE = False
LAST_EXEC_NS = None

_cache = {}


def _build(with_mask: bool):
    import concourse.tile as tile
    from concourse import bacc, mybir
    from contextlib import ExitStack

    F32 = mybir.dt.float32
    BF16 = mybir.dt.bfloat16
    AF = mybir.ActivationFunctionType
    ALU = mybir.AluOpType

    nc = bacc.Bacc("TRN2", target_bir_lowering=False, debug=False,
                   enable_asserts=False, num_devices=N_CORES)

    xT_d = nc.dram_tensor("xT", [E, NT], F32, kind="ExternalInput")
    xTb_d = nc.dram_tensor("xTb", [E, NT], BF16, kind="ExternalInput")
    qw_d = nc.dram_tensor("qw", [MC_E, 128, KC, 128], BF16, kind="ExternalInput")
    kw_d = nc.dram_tensor("kw", [MC_E, 128, KC, 128], BF16, kind="ExternalInput")
    vw_d = nc.dram_tensor("vw", [KC, 128, E], BF16, kind="ExternalInput")
    ow_d = nc.dram_tensor("ow", [MC_E, 128, KC, 128], BF16, kind="ExternalInput")
    f1w_d = nc.dram_tensor("f1w", [MC_I, 128, KC, 128], BF16, kind="ExternalInput")
    f2w_d = nc.dram_tensor("f2w", [MC_E, 128, MC_I, 128], BF16, kind="ExternalInput")
    qb_d = nc.dram_tensor("qb", [128, MC_E], F32, kind="ExternalInput")
    kb_d = nc.dram_tensor("kb", [128, MC_E], F32, kind="ExternalInput")
    vb_d = nc.dram_tensor("vb", [1, E], F32, kind="ExternalInput")
    ob_d = nc.dram_tensor("ob", [128, MC_E], F32, kind="ExternalInput")
    f1b_d = nc.dram_tensor("f1b", [128, MC_I], F32, kind="ExternalInput")
    f2b_d = nc.dram_tensor("f2b", [128, MC_E], F32, kind="ExternalInput")
    mskT_d = None
    if with_mask:
        mskT_d = nc.dram_tensor("mskT", [B_LOC, S, S], F32, kind="ExternalInput")
    outT_d = nc.dram_tensor("outT", [E, NT], F32, kind="ExternalOutput")

    with tile.TileContext(nc) as tc, ExitStack() as top:
        consts = top.enter_context(tc.tile_pool(name="consts", bufs=1))

        ones_col = consts.tile([128, 1], BF16)
        nc.vector.memset(ones_col[:], 1.0)
        ones_row = consts.tile([1, 128], BF16)
        nc.vector.memset(ones_row[:], 1.0)
        eps_t = consts.tile([1, 1], F32)
        nc.vector.memset(eps_t[:], EPS)
        qb_sb = consts.tile([128, MC_E], F32)
        nc.sync.dma_start(out=qb_sb[:], in_=qb_d[:])
        kb_sb = consts.tile([128, MC_E], F32)
        nc.sync.dma_start(out=kb_sb[:], in_=kb_d[:])
        ob_sb = consts.tile([128, MC_E], F32)
        nc.sync.dma_start(out=ob_sb[:], in_=ob_d[:])
        f2b_sb = consts.tile([128, MC_E], F32)
        nc.sync.dma_start(out=f2b_sb[:], in_=f2b_d[:])
        f1b_sb = consts.tile([128, MC_I], F32)
        nc.sync.dma_start(out=f1b_sb[:], in_=f1b_d[:])
        vb_sb = consts.tile([128, E], F32)
        nc.sync.dma_start(out=vb_sb[:], in_=vb_d[0:1, :].to_broadcast((128, E)))

        def emit_ln(ph, src_ap, srcbf_ap, out_pool, sfx):
            """Per-batch column LayerNorm over the feature (partition) dim.
            src_ap(k, b) -> [128, S] f32 AP; srcbf_ap(k, b) -> [128, S] bf16
            AP.  Returns {(k, b): [128, S] bf16 tile} of (x - mu) * rstd
            (LN scale/bias folded into downstream weights host-side)."""
            lntmp = ph.enter_context(tc.tile_pool(name=f"lntmp{sfx}", bufs=3))
            sqp = ph.enter_context(tc.tile_pool(name=f"sqp{sfx}", bufs=3))
            rows = ph.enter_context(tc.tile_pool(name=f"rows{sfx}", bufs=8))
            pstat = ph.enter_context(
                tc.tile_pool(name=f"pstat{sfx}", bufs=2, space="PSUM"))
            pbc = ph.enter_context(
                tc.tile_pool(name=f"pbc{sfx}", bufs=2, space="PSUM"))
            outs = {}
            for b in range(B_LOC):
                ps_sum = pstat.tile([1, 512], F32, name="ps_sum", tag="stat")
                ps_sq = pstat.tile([1, 512], F32, name="ps_sq", tag="stat")
                for k in range(KC):
                    xb = srcbf_ap(k, b)
                    sq = sqp.tile([128, S], BF16, name="sq", tag="sq")
                    nc.scalar.activation(out=sq[:], in_=xb, func=AF.Square)
                    nc.tensor.matmul(ps_sum[0:1, 0:S], ones_col[:], xb,
                                     start=(k == 0), stop=(k == KC - 1))
                    nc.tensor.matmul(ps_sq[0:1, 0:S], ones_col[:], sq[:],
                                     start=(k == 0), stop=(k == KC - 1))
                musq = rows.tile([1, S], F32, name="musq", tag="row")
                nc.scalar.activation(out=musq[0:1, :], in_=ps_sum[0:1, 0:S],
                                     func=AF.Square, scale=-1.0 / E)
                muneg_b = rows.tile([1, S], BF16, name="muneg_b", tag="row")
                nc.scalar.mul(out=muneg_b[0:1, :], in_=ps_sum[0:1, 0:S],
                              mul=-1.0 / E)
                var = rows.tile([1, S], F32, name="var", tag="row")
                nc.vector.scalar_tensor_tensor(
                    out=var[0:1, :], in0=ps_sq[0:1, 0:S], scalar=1.0 / E,
                    in1=musq[0:1, :], op0=ALU.mult, op1=ALU.subtract)
                sd = rows.tile([1, S], F32, name="sd", tag="row")
                nc.scalar.activation(out=sd[0:1, :], in_=var[0:1, :],
                                     func=AF.Sqrt, bias=eps_t[0:1, 0:1])
                rstd = rows.tile([1, S], F32, name="rstd", tag="row")
                nc.vector.reciprocal_approx_fast(out=rstd[0:1, :],
                                                 in_=sd[0:1, :])
                rstd_b = rows.tile([1, S], BF16, name="rstd_b", tag="row")
                nc.vector.tensor_copy(out=rstd_b[0:1, :], in_=rstd[0:1, :])
                psA = pbc.tile([128, 512], F32, name="psA", tag="bc")
                psB = pbc.tile([128, 512], F32, name="psB", tag="bc")
                nc.tensor.matmul(psA[:, 0:S], ones_row[0:1, :],
                                 rstd_b[0:1, :], start=True, stop=True)
                nc.tensor.matmul(psB[:, 0:S], ones_row[0:1, :],
                                 muneg_b[0:1, :], start=True, stop=True)
                for k in range(KC):
                    tmp = lntmp.tile([128, S], F32, name="tmp", tag="ap")
                    nc.vector.tensor_add(out=tmp[:], in0=src_ap(k, b),
                                         in1=psB[:, 0:S])
                    o = out_pool.tile([128, S], BF16, name="lno", tag="lno")
                    nc.vector.tensor_mul(out=o[:], in0=tmp[:], in1=psA[:, 0:S])
                    outs[(k, b)] = o
            return outs

        with tc.tile_pool(name="xt", bufs=KC * B_LOC) as xt_p:
            with tc.tile_pool(name="ctxT", bufs=MC_E) as ctx_p:
                ctxT = [ctx_p.tile([128, NT], BF16, tag="ctxT", name="ctxT")
                        for _ in range(MC_E)]

                # ============= LN1 / V / QK+attention ====================
                with tc.tile_pool(name="xln1", bufs=KC * B_LOC) as xln1_p:
                    with ExitStack() as ln1_ph:
                        xtb_p = ln1_ph.enter_context(
                            tc.tile_pool(name="xtb", bufs=KC * B_LOC))
                        xtb = {}
                        for b in range(B_LOC):
                            for k in range(KC):
                                tb = xtb_p.tile([128, S], BF16, name="xtb",
                                                tag="xtb")
                                nc.sync.dma_start(
                                    out=tb[:],
                                    in_=xTb_d[k * 128:(k + 1) * 128,
                                              b * S:(b + 1) * S])
                                xtb[(k, b)] = tb
                        xln1 = emit_ln(ln1_ph,
                                       lambda k, b: xtb[(k, b)][:],
                                       lambda k, b: xtb[(k, b)][:],
                                       xln1_p, "1")

                        # ============= V projection ======================
                        v_p = ln1_ph.enter_context(
                            tc.tile_pool(name="vpool", bufs=3 * B_LOC))
                        v_tiles = {}
                        with ExitStack() as ph:
                            vw_p = ph.enter_context(
                                tc.tile_pool(name="vw", bufs=1))
                            ppv = ph.enter_context(
                                tc.tile_pool(name="ppv", bufs=2, space="PSUM"))
                            vw_sb = vw_p.tile([128, KC, E], BF16)
                            for k in range(KC):
                                nc.sync.dma_start(out=vw_sb[:, k, :],
                                                  in_=vw_d[k, :, :])
                            for b in range(B_LOC):
                                for jc, (j0, jcs) in enumerate(JC):
                                    ps = ppv.tile([128, 2, 512], F32,
                                                  name="vps", tag="vps")
                                    for n in range(2):
                                        for k in range(KC):
                                            nc.tensor.matmul(
                                                ps[0:jcs, n, :],
                                                xln1[(k, b)][:, j0:j0 + jcs],
                                                vw_sb[:, k,
                                                      n * 512:(n + 1) * 512],
                                                start=(k == 0),
                                                stop=(k == KC - 1))
                                    # [tok, H, 128]: cols 0:64 ones, cols
                                    # 64:128 V -> ctx matmul replicates the
                                    # softmax sums across partitions 0:64
                                    # (base 0: custom-DVE recip needs it).
                                    vt = v_p.tile([128, H, 128], BF16,
                                                  name="vt", tag="vt")
                                    nc.vector.tensor_add(
                                        out=vt[0:jcs, :, 64:128],
                                        in0=ps[0:jcs, :, :],
                                        in1=vb_sb[0:jcs, :])
                                    nc.vector.memset(vt[:, :, 0:64], 1.0)
                                    v_tiles[(b, jc)] = vt

                    # ========= Q/K + attention (per head-pair chunk) =====
                    with ExitStack() as ph:
                        qt_p = ph.enter_context(tc.tile_pool(name="qt", bufs=2))
                        kt_p = ph.enter_context(tc.tile_pool(name="kt", bufs=2))
                        wqk_p = ph.enter_context(
                            tc.tile_pool(name="wqk", bufs=3))
                        e_p = ph.enter_context(tc.tile_pool(name="ep", bufs=6))
                        rs_p = ph.enter_context(tc.tile_pool(name="rsp", bufs=3))
                        if with_mask:
                            msk_p = ph.enter_context(
                                tc.tile_pool(name="mskp", bufs=3 * B_LOC))
                        pp2 = ph.enter_context(
                            tc.tile_pool(name="pp2", bufs=1, space="PSUM"))
                        psp = ph.enter_context(
                            tc.tile_pool(name="psp", bufs=2, space="PSUM"))
                        pcp = ph.enter_context(
                            tc.tile_pool(name="pcp", bufs=1, space="PSUM"))
                        if with_mask:
                            msk = {}
                            for b in range(B_LOC):
                                for jc, (j0, jcs) in enumerate(JC):
                                    mt = msk_p.tile([128, S], F32, name="mt",
                                                    tag="mt")
                                    nc.sync.dma_start(
                                        out=mt[0:jcs, :],
                                        in_=mskT_d[b, j0:j0 + jcs, :])
                                    msk[(b, jc)] = mt

                        for ec in range(MC_E):
                            qkt = []
                            for (w_d, b_sb, opool) in (
                                    (qw_d, qb_sb, qt_p),
                                    (kw_d, kb_sb, kt_p)):
                                wt = wqk_p.tile([128, KC, 128], BF16,
                                                name="wqk", tag="wqk")
                                nc.sync.dma_start(out=wt[:],
                                                  in_=w_d[ec, :, :, :])
                                ot = opool.tile([128, NT], BF16,
                                                name="qk", tag="qk")
                                for half in range(2):
                                    ps = pp2.tile([128, 2, 512], F32,
                                                  name="pqk", tag="pqk")
                                    for bb in range(2):
                                        b = half * 2 + bb
                                        for k in range(KC):
                                            nc.tensor.matmul(
                                                ps[:, bb, 0:S],
                                                wt[:, k, :],
                                                xln1[(k, b)][:],
                                                start=(k == 0),
                                                stop=(k == KC - 1))
                                    nc.vector.tensor_scalar_add(
                                        out=ot[:, half * 2 * S:
                                               (half + 1) * 2 * S],
                                        in0=ps[:, :, 0:S],
                                        scalar1=b_sb[:, ec:ec + 1])
                                qkt.append(ot)
                            qte, kte = qkt

                            for b in range(B_LOC):
                                ets = []
                                for jc, (j0, jcs) in enumerate(JC):
                                    sp = psp.tile([128, 2, 512], F32,
                                                  name="sp", tag="sp")
                                    for hi in range(2):
                                        p0 = hi * 64
                                        nc.tensor.matmul(
                                            sp[0:jcs, hi, 0:S],
                                            kte[p0:p0 + 64,
                                                b * S + j0: b * S + j0 + jcs],
                                            qte[p0:p0 + 64,
                                                b * S:(b + 1) * S],
                                            start=True, stop=True)
                                    if with_mask:
                                        for hi in range(2):
                                            nc.vector.tensor_add(
                                                out=sp[0:jcs, hi, 0:S],
                                                in0=sp[0:jcs, hi, 0:S],
                                                in1=msk[(b, jc)][0:jcs, :])
                                    et = e_p.tile([128, 2, S], BF16,
                                                  name="et", tag="et")
                                    nc.scalar.activation(
                                        out=et[0:jcs, :, :],
                                        in_=sp[0:jcs, :, 0:S], func=AF.Exp)
                                    ets.append(et)
                                cp = pcp.tile([128, 2, 512], F32,
                                              name="cp", tag="cp")
                                for hi in range(2):
                                    h = 2 * ec + hi
                                    for jc, (j0, jcs) in enumerate(JC):
                                        nc.tensor.matmul(
                                            cp[0:128, hi, 0:S],
                                            v_tiles[(b, jc)][0:jcs, h, :],
                                            ets[jc][0:jcs, hi, :],
                                            start=(jc == 0), stop=(jc == 2))
                                rst = rs_p.tile([64, 2, S], F32,
                                                name="rst", tag="rst")
                                nc.vector.reciprocal_approx_fast(
                                    out=rst[0:64, :, :],
                                    in_=cp[0:64, :, 0:S])
                                for hi in range(2):
                                    nc.vector.tensor_mul(
                                        out=ctxT[ec][hi * 64:hi * 64 + 64,
                                                     b * S:(b + 1) * S],
                                        in0=cp[64:128, hi, 0:S],
                                        in1=rst[0:64, hi, :])

                # residual x^T (f32) loads late - DMA is idle by now
                xt = {}
                for b in range(B_LOC):
                    for k in range(KC):
                        t = xt_p.tile([128, S], F32, name="xt", tag="xt")
                        nc.sync.dma_start(
                            out=t[:],
                            in_=xT_d[k * 128:(k + 1) * 128,
                                     b * S:(b + 1) * S])
                        xt[(k, b)] = t

                # xln1 closed; right-side long-lived pools
                ht_p = top.enter_context(
                    tc.tile_pool(name="ht", bufs=KC, side="right"))
                f1o_p = top.enter_context(
                    tc.tile_pool(name="f1o", bufs=MC_I, side="right"))

                # ============= out projection + residual =================
                ht = []
                with ExitStack() as ph:
                    wo_p = ph.enter_context(tc.tile_pool(name="wo", bufs=3))
                    ppo = ph.enter_context(
                        tc.tile_pool(name="ppo", bufs=2, space="PSUM"))
                    for m in range(MC_E):
                        wt = wo_p.tile([128, KC, 128], BF16, name="wo",
                                       tag="wo")
                        nc.sync.dma_start(out=wt[:], in_=ow_d[m, :, :, :])
                        ps = ppo.tile([128, B_LOC, 512], F32, name="po",
                                      tag="po")
                        for b in range(B_LOC):
                            for k in range(KC):
                                nc.tensor.matmul(
                                    ps[:, b, 0:S], wt[:, k, :],
                                    ctxT[k][:, b * S:(b + 1) * S],
                                    start=(k == 0), stop=(k == KC - 1))
                        o = ht_p.tile([128, NT], F32, name="ht", tag="ht")
                        for b in range(B_LOC):
                            nc.vector.scalar_tensor_tensor(
                                out=o[:, b * S:(b + 1) * S],
                                in0=ps[:, b, 0:S],
                                scalar=ob_sb[:, m:m + 1],
                                in1=xt[(m, b)][:],
                                op0=ALU.add, op1=ALU.add)
                        ht.append(o)
            # ctxT closed
        # xt closed

        # ================= LN2 + MLP =====================================
        with tc.tile_pool(name="xln2", bufs=KC * B_LOC) as xln2_p:
            f1o = []
            with ExitStack() as ln2_ph:
                htb_p = ln2_ph.enter_context(tc.tile_pool(name="htb", bufs=KC))
                htb = []
                for k in range(KC):
                    hb = htb_p.tile([128, NT], BF16, name="htb", tag="htb")
                    nc.vector.tensor_copy(out=hb[:], in_=ht[k][:])
                    htb.append(hb)
                xln2 = emit_ln(
                    ln2_ph,
                    lambda k, b: ht[k][:, b * S:(b + 1) * S],
                    lambda k, b: htb[k][:, b * S:(b + 1) * S],
                    xln2_p, "2")
                wf1_p = ln2_ph.enter_context(tc.tile_pool(name="wf1", bufs=3))
                ppf1 = ln2_ph.enter_context(
                    tc.tile_pool(name="ppf1", bufs=2, space="PSUM"))
                for m in range(MC_I):
                    wt = wf1_p.tile([128, KC, 128], BF16, name="wf1",
                                    tag="wf1")
                    nc.sync.dma_start(out=wt[:], in_=f1w_d[m, :, :, :])
                    o = f1o_p.tile([128, NT], BF16, name="f1o", tag="f1o")
                    for half in range(2):
                        ps = ppf1.tile([128, 2, 512], F32, name="pf1",
                                       tag="pf1")
                        for bb in range(2):
                            b = half * 2 + bb
                            for k in range(KC):
                                nc.tensor.matmul(
                                    ps[:, bb, 0:S], wt[:, k, :],
                                    xln2[(k, b)][:],
                                    start=(k == 0), stop=(k == KC - 1))
                        nc.scalar.activation(
                            out=o[:, half * 2 * S:(half + 1) * 2 * S],
                            in_=ps[:, :, 0:S],
                            func=AF.Gelu_apprx_tanh,
                            bias=f1b_sb[:, m:m + 1])
                    f1o.append(o)

        with ExitStack() as ph:
            wf2_p = ph.enter_context(tc.tile_pool(name="wf2", bufs=2))
            ppf2 = ph.enter_context(
                tc.tile_pool(name="ppf2", bufs=2, space="PSUM"))
            out_p = ph.enter_context(tc.tile_pool(name="outp", bufs=3))
            for m in range(MC_E):
                wt = wf2_p.tile([128, MC_I, 128], BF16, name="wf2", tag="wf2")
                nc.sync.dma_start(out=wt[:], in_=f2w_d[m, :, :, :])
                ps = ppf2.tile([128, B_LOC, 512], F32, name="pf2", tag="pf2")
                for b in range(B_LOC):
                    for k in range(MC_I):
                        nc.tensor.matmul(
                            ps[:, b, 0:S], wt[:, k, :],
                            f1o[k][:, b * S:(b + 1) * S],
                            start=(k == 0), stop=(k == MC_I - 1))
                o = out_p.tile([128, NT], F32, name="oo", tag="oo")
                nc.vector.scalar_tensor_tensor(
                    out=o[:], in0=ps[:, :, 0:S], scalar=f2b_sb[:, m:m + 1],
                    in1=ht[m][:], op0=ALU.add, op1=ALU.add)
                nc.sync.dma_start(out=outT_d[m * 128:(m + 1) * 128, :],
                                  in_=o[:])

    nc.compile()
    return nc


def _pack_lhsT(W):
    """W [M, K] (out, in) -> [M/128, 128, K/128, 128] bf16 with
    [m, p, k, j] = W[m*128+j, k*128+p] (lhsT tiles, partition = K)."""
    W = np.asarray(W, np.float32)
    M, K = W.shape
    A = W.reshape(M // 128, 128, K // 128, 128)
    return np.ascontiguousarray(A.transpose(0, 3, 2, 1)).astype(ml_dtypes.bfloat16)


def _pack_pbias(b):
    """b [M] -> [128, M/128] f32 per-partition bias columns."""
    return np.ascontiguousarray(np.asarray(b, np.float32).reshape(-1, 128).T)


def kernel(hidden_states, attention_mask, causal_attention_mask,
           ln1_w, ln1_b, q_w, q_b, k_w, k_b, v_w, v_b, o_w, o_b,
           ln2_w, ln2_b, fc1_w, fc1_b, fc2_w, fc2_b):
    global LAST_EXEC_NS
    from concourse.bass_utils import run_bass_kernel_spmd

    hs = np.asarray(hidden_states, np.float32)
    msk = (np.asarray(attention_mask, np.float32)
           + np.asarray(causal_attention_mask, np.float32))
    with_mask = bool(np.any(msk))

    ln1_w = np.asarray(ln1_w, np.float32); ln1_b = np.asarray(ln1_b, np.float32)
    ln2_w = np.asarray(ln2_w, np.float32); ln2_b = np.asarray(ln2_b, np.float32)
    q_w = np.asarray(q_w, np.float32); q_b = np.asarray(q_b, np.float32)
    k_w = np.asarray(k_w, np.float32); k_b = np.asarray(k_b, np.float32)
    v_w = np.asarray(v_w, np.float32); v_b = np.asarray(v_b, np.float32)
    o_w = np.asarray(o_w, np.float32); o_b = np.asarray(o_b, np.float32)
    fc1_w = np.asarray(fc1_w, np.float32); fc1_b = np.asarray(fc1_b, np.float32)
    fc2_w = np.asarray(fc2_w, np.float32); fc2_b = np.asarray(fc2_b, np.float32)

    scale = D ** -0.5
    # fold LN1 scale/bias into Q/K/V, and the softmax scale into Q
    qw_eff = (q_w * ln1_w[None, :]) * scale
    qb_eff = (q_b + q_w @ ln1_b) * scale
    kw_eff = k_w * ln1_w[None, :]
    kb_eff = k_b + k_w @ ln1_b
    vw_eff = v_w * ln1_w[None, :]
    vb_eff = v_b + v_w @ ln1_b
    # fold LN2 into fc1
    f1w_eff = fc1_w * ln2_w[None, :]
    f1b_eff = fc1_b + fc1_w @ ln2_b

    base = {
        "qw": _pack_lhsT(qw_eff),
        "kw": _pack_lhsT(kw_eff),
        "vw": np.ascontiguousarray(
            vw_eff.T.reshape(KC, 128, E)).astype(ml_dtypes.bfloat16),
        "ow": _pack_lhsT(o_w),
        "f1w": _pack_lhsT(f1w_eff),
        "f2w": _pack_lhsT(fc2_w),
        "qb": _pack_pbias(qb_eff),
        "kb": _pack_pbias(kb_eff),
        "vb": np.ascontiguousarray(vb_eff[None, :].astype(np.float32)),
        "ob": _pack_pbias(o_b),
        "f1b": _pack_pbias(f1b_eff),
        "f2b": _pack_pbias(fc2_b),
    }

    key = with_mask
    if key not in _cache:
        _cache[key] = _build(with_mask)
    nc = _cache[key]

    in_maps = []
    for c in range(N_CORES):
        x = hs[c * B_LOC:(c + 1) * B_LOC].reshape(NT, E).T
        m = dict(base)
        m["xT"] = np.ascontiguousarray(x)
        m["xTb"] = np.ascontiguousarray(x).astype(ml_dtypes.bfloat16)
        if with_mask:
            m["mskT"] = np.ascontiguousarray(
                msk[c * B_LOC:(c + 1) * B_LOC, 0].transpose(0, 2, 1))
        in_maps.append(m)

    res = run_bass_kernel_spmd(nc, in_maps, core_ids=list(range(N_CORES)),
                               trace=TRACE)
    LAST_EXEC_NS = res.exec_time_ns

    outs = []
    for c in range(N_CORES):
        oT = res.results[c]["outT"]          # [E, NT] f32
        outs.append(np.ascontiguousarray(oT.T).reshape(B_LOC, S, E))
    return np.concatenate(outs, axis=0)


# revision 23
# speedup vs baseline: 1.0731x; 1.0085x over previous
"""Trainium2 Bass kernel for a CLIP encoder layer (B=32, S=257, E=1024, H=16, I=4096).

Strategy: data-parallel over batch across 8 NeuronCores (4 batch elements per
core), no collectives.  Per-core compute is done feature-major ([E, tokens])
so projection matmuls need no on-device transposes:

  - LayerNorm: column stats via PE ones-matmuls (f32r), normalization applied
    with two DVE passes; LN scale/bias are folded into the projection weights
    on the host.
  - Q/K/O/fc1/fc2: weight-stationary matmuls (lhsT = W^T packed on host,
    bf16), fp32 PSUM accumulation, N=257 (one batch element) moving slices.
  - V: activation-stationary -> token-major [tok, H, 65] with a ones column,
    so the softmax denominators fall out of the ctx matmul for free.
  - Attention: scores computed transposed (scores^T[j, i]) so softmax reduces
    over the partition dim via the ctx matmul; exp on ACT straight from PSUM;
    two heads (D=64) packed per PE pass at partition bases 0/64.
"""

import numpy as np
import ml_dtypes

B, S, E, H, D, II = 32, 257, 1024, 16, 64, 4096
N_CORES = 8
B_LOC = B // N_CORES          # 4
NT = B_LOC * S                # 1028
KC = E // 128                 # 8
MC_E = E // 128               # 8
MC_I = II // 128              # 32
EPS = 1e-5

# token slices within NT, used for f32r matmuls (N>=256 except the 4-tail)
LN_SLICES = [(0, 512), (512, 1024), (1024, NT)]
# j-chunks of one batch element's 257 keys
JC = [(0, 128), (128, 128), (256, 1)]

TRACE = False
LAST_EXEC_NS = None

_cache = {}


def _build(with_mask: bool, with_vbias: bool):
    import concourse.tile as tile
    from concourse import bacc, mybir
    from contextlib import ExitStack

    F32 = mybir.dt.float32
    BF16 = mybir.dt.bfloat16
    AF = mybir.ActivationFunctionType
    ALU = mybir.AluOpType

    nc = bacc.Bacc("TRN2", target_bir_lowering=False, debug=False,
                   enable_asserts=False, num_devices=N_CORES)

    xT_d = nc.dram_tensor("xT", [E, NT], F32, kind="ExternalInput")
    xTb_d = nc.dram_tensor("xTb", [E, NT], BF16, kind="ExternalInput")
    qw_d = nc.dram_tensor("qw", [MC_E, 128, KC, 128], BF16, kind="ExternalInput")
    kw_d = nc.dram_tensor("kw", [MC_E, 128, KC, 128], BF16, kind="ExternalInput")
    vw_d = nc.dram_tensor("vw", [KC, 128, E], BF16, kind="ExternalInput")
    ow_d = nc.dram_tensor("ow", [MC_E, 128, KC, 128], BF16, kind="ExternalInput")
    f1w_d = nc.dram_tensor("f1w", [MC_I, 128, KC, 128], BF16, kind="ExternalInput")
    f2w_d = nc.dram_tensor("f2w", [MC_E, 128, MC_I, 128], BF16, kind="ExternalInput")
    qb_d = nc.dram_tensor("qb", [128, MC_E], F32, kind="ExternalInput")
    kb_d = nc.dram_tensor("kb", [128, MC_E], F32, kind="ExternalInput")
    vb_d = nc.dram_tensor("vb", [1, E], F32, kind="ExternalInput")
    ob_d = nc.dram_tensor("ob", [128, MC_E], F32, kind="ExternalInput")
    f1b_d = nc.dram_tensor("f1b", [128, MC_I], F32, kind="ExternalInput")
    f2b_d = nc.dram_tensor("f2b", [128, MC_E], F32, kind="ExternalInput")
    mskT_d = None
    if with_mask:
        mskT_d = nc.dram_tensor("mskT", [B_LOC, S, S], F32, kind="ExternalInput")
    outT_d = nc.dram_tensor("outT", [E, NT], F32, kind="ExternalOutput")

    with tile.TileContext(nc) as tc, ExitStack() as top:
        consts = top.enter_context(tc.tile_pool(name="consts", bufs=1))

        ones_col = consts.tile([128, 1], BF16)
        nc.vector.memset(ones_col[:], 1.0)
        ones_row = consts.tile([1, 128], BF16)
        nc.vector.memset(ones_row[:], 1.0)
        eps_t = consts.tile([1, 1], F32)
        nc.vector.memset(eps_t[:], EPS)
        qb_sb = consts.tile([128, MC_E], F32)
        nc.sync.dma_start(out=qb_sb[:], in_=qb_d[:])
        kb_sb = consts.tile([128, MC_E], F32)
        nc.sync.dma_start(out=kb_sb[:], in_=kb_d[:])
        ob_sb = consts.tile([128, MC_E], F32)
        nc.sync.dma_start(out=ob_sb[:], in_=ob_d[:])
        f2b_sb = consts.tile([128, MC_E], F32)
        nc.sync.dma_start(out=f2b_sb[:], in_=f2b_d[:])
        f1b_sb = consts.tile([128, MC_I], F32)
        nc.sync.dma_start(out=f1b_sb[:], in_=f1b_d[:])
        vb_sb = consts.tile([128, E], F32)
        nc.sync.dma_start(out=vb_sb[:], in_=vb_d[0:1, :].to_broadcast((128, E)))

        def emit_ln(ph, src_ap, srcbf_ap, out_pool, sfx):
            """Per-batch column LayerNorm over the feature (partition) dim.
            src_ap(k, b) -> [128, S] f32 AP; srcbf_ap(k, b) -> [128, S] bf16
            AP.  Returns {(k, b): [128, S] bf16 tile} of (x - mu) * rstd
            (LN scale/bias folded into downstream weights host-side)."""
            lntmp = ph.enter_context(tc.tile_pool(name=f"lntmp{sfx}", bufs=3))
            sqp = ph.enter_context(tc.tile_pool(name=f"sqp{sfx}", bufs=3))
            rows = ph.enter_context(tc.tile_pool(name=f"rows{sfx}", bufs=8))
            pstat = ph.enter_context(
                tc.tile_pool(name=f"pstat{sfx}", bufs=2, space="PSUM"))
            pbc = ph.enter_context(
                tc.tile_pool(name=f"pbc{sfx}", bufs=2, space="PSUM"))
            outs = {}
            for b in range(B_LOC):
                ps_sum = pstat.tile([1, 512], F32, name="ps_sum", tag="stat")
                ps_sq = pstat.tile([1, 512], F32, name="ps_sq", tag="stat")
                for k in range(KC):
                    xb = srcbf_ap(k, b)
                    sq = sqp.tile([128, S], BF16, name="sq", tag="sq")
                    nc.scalar.activation(out=sq[:], in_=xb, func=AF.Square)
                    nc.tensor.matmul(ps_sum[0:1, 0:S], ones_col[:], xb,
                                     start=(k == 0), stop=(k == KC - 1))
                    nc.tensor.matmul(ps_sq[0:1, 0:S], ones_col[:], sq[:],
                                     start=(k == 0), stop=(k == KC - 1))
                musq = rows.tile([1, S], F32, name="musq", tag="row")
                nc.scalar.activation(out=musq[0:1, :], in_=ps_sum[0:1, 0:S],
                                     func=AF.Square, scale=-1.0 / E)
                muneg_b = rows.tile([1, S], BF16, name="muneg_b", tag="row")
                nc.scalar.mul(out=muneg_b[0:1, :], in_=ps_sum[0:1, 0:S],
                              mul=-1.0 / E)
                var = rows.tile([1, S], F32, name="var", tag="row")
                nc.vector.scalar_tensor_tensor(
                    out=var[0:1, :], in0=ps_sq[0:1, 0:S], scalar=1.0 / E,
                    in1=musq[0:1, :], op0=ALU.mult, op1=ALU.subtract)
                sd = rows.tile([1, S], F32, name="sd", tag="row")
                nc.scalar.activation(out=sd[0:1, :], in_=var[0:1, :],
                                     func=AF.Sqrt, bias=eps_t[0:1, 0:1])
                rstd = rows.tile([1, S], F32, name="rstd", tag="row")
                nc.vector.reciprocal_approx_fast(out=rstd[0:1, :],
                                                 in_=sd[0:1, :])
                rstd_b = rows.tile([1, S], BF16, name="rstd_b", tag="row")
                nc.vector.tensor_copy(out=rstd_b[0:1, :], in_=rstd[0:1, :])
                psA = pbc.tile([128, 512], F32, name="psA", tag="bc")
                psB = pbc.tile([128, 512], F32, name="psB", tag="bc")
                nc.tensor.matmul(psA[:, 0:S], ones_row[0:1, :],
                                 rstd_b[0:1, :], start=True, stop=True)
                nc.tensor.matmul(psB[:, 0:S], ones_row[0:1, :],
                                 muneg_b[0:1, :], start=True, stop=True)
                for k in range(KC):
                    tmp = lntmp.tile([128, S], F32, name="tmp", tag="ap")
                    nc.vector.tensor_add(out=tmp[:], in0=src_ap(k, b),
                                         in1=psB[:, 0:S])
                    o = out_pool.tile([128, S], BF16, name="lno", tag="lno")
                    nc.vector.tensor_mul(out=o[:], in0=tmp[:], in1=psA[:, 0:S])
                    outs[(k, b)] = o
            return outs

        with tc.tile_pool(name="xt", bufs=KC * B_LOC) as xt_p:
            with tc.tile_pool(name="ctxT", bufs=MC_E) as ctx_p:
                ctxT = [ctx_p.tile([128, NT], BF16, tag="ctxT", name="ctxT")
                        for _ in range(MC_E)]

                # ============= LN1 / V / QK+attention ====================
                with tc.tile_pool(name="xln1", bufs=KC * B_LOC) as xln1_p:
                    with ExitStack() as ln1_ph:
                        xtb_p = ln1_ph.enter_context(
                            tc.tile_pool(name="xtb", bufs=KC * B_LOC))
                        xtb = {}
                        for b in range(B_LOC):
                            for k in range(KC):
                                tb = xtb_p.tile([128, S], BF16, name="xtb",
                                                tag="xtb")
                                nc.sync.dma_start(
                                    out=tb[:],
                                    in_=xTb_d[k * 128:(k + 1) * 128,
                                              b * S:(b + 1) * S])
                                xtb[(k, b)] = tb
                        xln1 = emit_ln(ln1_ph,
                                       lambda k, b: xtb[(k, b)][:],
                                       lambda k, b: xtb[(k, b)][:],
                                       xln1_p, "1")

                        # ============= V projection ======================
                        v_p = ln1_ph.enter_context(
                            tc.tile_pool(name="vpool", bufs=3 * B_LOC))
                        v_tiles = {}
                        with ExitStack() as ph:
                            vw_p = ph.enter_context(
                                tc.tile_pool(name="vw", bufs=1))
                            ppv = ph.enter_context(
                                tc.tile_pool(name="ppv", bufs=2, space="PSUM"))
                            vw_sb = vw_p.tile([128, KC, E], BF16)
                            for k in range(KC):
                                nc.sync.dma_start(out=vw_sb[:, k, :],
                                                  in_=vw_d[k, :, :])
                            for b in range(B_LOC):
                                for jc, (j0, jcs) in enumerate(JC):
                                    ps = ppv.tile([128, 2, 512], F32,
                                                  name="vps", tag="vps")
                                    for n in range(2):
                                        for k in range(KC):
                                            nc.tensor.matmul(
                                                ps[0:jcs, n, :],
                                                xln1[(k, b)][:, j0:j0 + jcs],
                                                vw_sb[:, k,
                                                      n * 512:(n + 1) * 512],
                                                start=(k == 0),
                                                stop=(k == KC - 1))
                                    # [tok, H, 128]: cols 0:64 ones, cols
                                    # 64:128 V -> ctx matmul replicates the
                                    # softmax sums across partitions 0:64
                                    # (base 0: custom-DVE recip needs it).
                                    vt = v_p.tile([128, H, 128], BF16,
                                                  name="vt", tag="vt")
                                    if with_vbias:
                                        nc.vector.tensor_add(
                                            out=vt[0:jcs, :, 64:128],
                                            in0=ps[0:jcs, :, :],
                                            in1=vb_sb[0:jcs, :])
                                    else:
                                        nc.scalar.copy(
                                            out=vt[0:jcs, :, 64:128],
                                            in_=ps[0:jcs, :, :])
                                    nc.gpsimd.memset(vt[:, :, 0:64], 1.0)
                                    v_tiles[(b, jc)] = vt

                    # ========= Q/K + attention (per head-pair chunk) =====
                    with ExitStack() as ph:
                        qt_p = ph.enter_context(tc.tile_pool(name="qt", bufs=2))
                        kt_p = ph.enter_context(tc.tile_pool(name="kt", bufs=2))
                        wqk_p = ph.enter_context(
                            tc.tile_pool(name="wqk", bufs=3))
                        e_p = ph.enter_context(tc.tile_pool(name="ep", bufs=6))
                        rs_p = ph.enter_context(tc.tile_pool(name="rsp", bufs=3))
                        if with_mask:
                            msk_p = ph.enter_context(
                                tc.tile_pool(name="mskp", bufs=3 * B_LOC))
                        pp2 = ph.enter_context(
                            tc.tile_pool(name="pp2", bufs=1, space="PSUM"))
                        psp = ph.enter_context(
                            tc.tile_pool(name="psp", bufs=2, space="PSUM"))
                        pcp = ph.enter_context(
                            tc.tile_pool(name="pcp", bufs=1, space="PSUM"))
                        if with_mask:
                            msk = {}
                            for b in range(B_LOC):
                                for jc, (j0, jcs) in enumerate(JC):
                                    mt = msk_p.tile([128, S], F32, name="mt",
                                                    tag="mt")
                                    nc.sync.dma_start(
                                        out=mt[0:jcs, :],
                                        in_=mskT_d[b, j0:j0 + jcs, :])
                                    msk[(b, jc)] = mt

                        for ec in range(MC_E):
                            qkt = []
                            for (w_d, b_sb, opool) in (
                                    (qw_d, qb_sb, qt_p),
                                    (kw_d, kb_sb, kt_p)):
                                wt = wqk_p.tile([128, KC, 128], BF16,
                                                name="wqk", tag="wqk")
                                nc.sync.dma_start(out=wt[:],
                                                  in_=w_d[ec, :, :, :])
                                ot = opool.tile([128, NT], BF16,
                                                name="qk", tag="qk")
                                for half in range(2):
                                    ps = pp2.tile([128, 2, 512], F32,
                                                  name="pqk", tag="pqk")
                                    for bb in range(2):
                                        b = half * 2 + bb
                                        for k in range(KC):
                                            nc.tensor.matmul(
                                                ps[:, bb, 0:S],
                                                wt[:, k, :],
                                                xln1[(k, b)][:],
                                                start=(k == 0),
                                                stop=(k == KC - 1))
                                    nc.vector.tensor_scalar_add(
                                        out=ot[:, half * 2 * S:
                                               (half + 1) * 2 * S],
                                        in0=ps[:, :, 0:S],
                                        scalar1=b_sb[:, ec:ec + 1])
                                qkt.append(ot)
                            qte, kte = qkt

                            for b in range(B_LOC):
                                ets = []
                                for jc, (j0, jcs) in enumerate(JC):
                                    sp = psp.tile([128, 2, 512], F32,
                                                  name="sp", tag="sp")
                                    for hi in range(2):
                                        p0 = hi * 64
                                        nc.tensor.matmul(
                                            sp[0:jcs, hi, 0:S],
                                            kte[p0:p0 + 64,
                                                b * S + j0: b * S + j0 + jcs],
                                            qte[p0:p0 + 64,
                                                b * S:(b + 1) * S],
                                            start=True, stop=True)
                                    if with_mask:
                                        for hi in range(2):
                                            nc.vector.tensor_add(
                                                out=sp[0:jcs, hi, 0:S],
                                                in0=sp[0:jcs, hi, 0:S],
                                                in1=msk[(b, jc)][0:jcs, :])
                                    et = e_p.tile([128, 2, S], BF16,
                                                  name="et", tag="et")
                                    nc.scalar.activation(
                                        out=et[0:jcs, :, :],
                                        in_=sp[0:jcs, :, 0:S], func=AF.Exp)
                                    ets.append(et)
                                cp = pcp.tile([128, 2, 512], F32,
                                              name="cp", tag="cp")
                                for hi in range(2):
                                    h = 2 * ec + hi
                                    for jc, (j0, jcs) in enumerate(JC):
                                        nc.tensor.matmul(
                                            cp[0:128, hi, 0:S],
                                            v_tiles[(b, jc)][0:jcs, h, :],
                                            ets[jc][0:jcs, hi, :],
                                            start=(jc == 0), stop=(jc == 2))
                                rst = rs_p.tile([64, 2, S], F32,
                                                name="rst", tag="rst")
                                nc.vector.reciprocal_approx_fast(
                                    out=rst[0:64, :, :],
                                    in_=cp[0:64, :, 0:S])
                                for hi in range(2):
                                    nc.vector.tensor_mul(
                                        out=ctxT[ec][hi * 64:hi * 64 + 64,
                                                     b * S:(b + 1) * S],
                                        in0=cp[64:128, hi, 0:S],
                                        in1=rst[0:64, hi, :])

                # residual x^T (f32) loads late - DMA is idle by now
                xt = {}
                for b in range(B_LOC):
                    for k in range(KC):
                        t = xt_p.tile([128, S], F32, name="xt", tag="xt")
                        nc.sync.dma_start(
                            out=t[:],
                            in_=xT_d[k * 128:(k + 1) * 128,
                                     b * S:(b + 1) * S])
                        xt[(k, b)] = t

                # xln1 closed; right-side long-lived pools
                ht_p = top.enter_context(
                    tc.tile_pool(name="ht", bufs=KC, side="right"))
                f1o_p = top.enter_context(
                    tc.tile_pool(name="f1o", bufs=MC_I, side="right"))

                # ============= out projection + residual =================
                ht = []
                with ExitStack() as ph:
                    wo_p = ph.enter_context(tc.tile_pool(name="wo", bufs=3))
                    ppo = ph.enter_context(
                        tc.tile_pool(name="ppo", bufs=2, space="PSUM"))
                    for m in range(MC_E):
                        wt = wo_p.tile([128, KC, 128], BF16, name="wo",
                                       tag="wo")
                        nc.sync.dma_start(out=wt[:], in_=ow_d[m, :, :, :])
                        ps = ppo.tile([128, B_LOC, 512], F32, name="po",
                                      tag="po")
                        for b in range(B_LOC):
                            for k in range(KC):
                                nc.tensor.matmul(
                                    ps[:, b, 0:S], wt[:, k, :],
                                    ctxT[k][:, b * S:(b + 1) * S],
                                    start=(k == 0), stop=(k == KC - 1))
                        o = ht_p.tile([128, NT], F32, name="ht", tag="ht")
                        for b in range(B_LOC):
                            nc.vector.scalar_tensor_tensor(
                                out=o[:, b * S:(b + 1) * S],
                                in0=ps[:, b, 0:S],
                                scalar=ob_sb[:, m:m + 1],
                                in1=xt[(m, b)][:],
                                op0=ALU.add, op1=ALU.add)
                        ht.append(o)
            # ctxT closed
        # xt closed

        # ================= LN2 + MLP =====================================
        with tc.tile_pool(name="xln2", bufs=KC * B_LOC) as xln2_p:
            f1o = []
            with ExitStack() as ln2_ph:
                htb_p = ln2_ph.enter_context(tc.tile_pool(name="htb", bufs=KC))
                htb = []
                for k in range(KC):
                    hb = htb_p.tile([128, NT], BF16, name="htb", tag="htb")
                    nc.vector.tensor_copy(out=hb[:], in_=ht[k][:])
                    htb.append(hb)
                xln2 = emit_ln(
                    ln2_ph,
                    lambda k, b: ht[k][:, b * S:(b + 1) * S],
                    lambda k, b: htb[k][:, b * S:(b + 1) * S],
                    xln2_p, "2")
                wf1_p = ln2_ph.enter_context(tc.tile_pool(name="wf1", bufs=3))
                ppf1 = ln2_ph.enter_context(
                    tc.tile_pool(name="ppf1", bufs=2, space="PSUM"))
                for m in range(MC_I):
                    wt = wf1_p.tile([128, KC, 128], BF16, name="wf1",
                                    tag="wf1")
                    nc.sync.dma_start(out=wt[:], in_=f1w_d[m, :, :, :])
                    o = f1o_p.tile([128, NT], BF16, name="f1o", tag="f1o")
                    for half in range(2):
                        ps = ppf1.tile([128, 2, 512], F32, name="pf1",
                                       tag="pf1")
                        for bb in range(2):
                            b = half * 2 + bb
                            for k in range(KC):
                                nc.tensor.matmul(
                                    ps[:, bb, 0:S], wt[:, k, :],
                                    xln2[(k, b)][:],
                                    start=(k == 0), stop=(k == KC - 1))
                        nc.scalar.activation(
                            out=o[:, half * 2 * S:(half + 1) * 2 * S],
                            in_=ps[:, :, 0:S],
                            func=AF.Gelu_apprx_tanh,
                            bias=f1b_sb[:, m:m + 1])
                    f1o.append(o)

        with ExitStack() as ph:
            wf2_p = ph.enter_context(tc.tile_pool(name="wf2", bufs=2))
            ppf2 = ph.enter_context(
                tc.tile_pool(name="ppf2", bufs=2, space="PSUM"))
            out_p = ph.enter_context(tc.tile_pool(name="outp", bufs=3))
            for m in range(MC_E):
                wt = wf2_p.tile([128, MC_I, 128], BF16, name="wf2", tag="wf2")
                nc.sync.dma_start(out=wt[:], in_=f2w_d[m, :, :, :])
                ps = ppf2.tile([128, B_LOC, 512], F32, name="pf2", tag="pf2")
                for b in range(B_LOC):
                    for k in range(MC_I):
                        nc.tensor.matmul(
                            ps[:, b, 0:S], wt[:, k, :],
                            f1o[k][:, b * S:(b + 1) * S],
                            start=(k == 0), stop=(k == MC_I - 1))
                o = out_p.tile([128, NT], F32, name="oo", tag="oo")
                nc.vector.scalar_tensor_tensor(
                    out=o[:], in0=ps[:, :, 0:S], scalar=f2b_sb[:, m:m + 1],
                    in1=ht[m][:], op0=ALU.add, op1=ALU.add)
                nc.sync.dma_start(out=outT_d[m * 128:(m + 1) * 128, :],
                                  in_=o[:])

    nc.compile()
    return nc


def _pack_lhsT(W):
    """W [M, K] (out, in) -> [M/128, 128, K/128, 128] bf16 with
    [m, p, k, j] = W[m*128+j, k*128+p] (lhsT tiles, partition = K)."""
    W = np.asarray(W, np.float32)
    M, K = W.shape
    A = W.reshape(M // 128, 128, K // 128, 128)
    return np.ascontiguousarray(A.transpose(0, 3, 2, 1)).astype(ml_dtypes.bfloat16)


def _pack_pbias(b):
    """b [M] -> [128, M/128] f32 per-partition bias columns."""
    return np.ascontiguousarray(np.asarray(b, np.float32).reshape(-1, 128).T)


def kernel(hidden_states, attention_mask, causal_attention_mask,
           ln1_w, ln1_b, q_w, q_b, k_w, k_b, v_w, v_b, o_w, o_b,
           ln2_w, ln2_b, fc1_w, fc1_b, fc2_w, fc2_b):
    global LAST_EXEC_NS
    from concourse.bass_utils import run_bass_kernel_spmd

    hs = np.asarray(hidden_states, np.float32)
    msk = (np.asarray(attention_mask, np.float32)
           + np.asarray(causal_attention_mask, np.float32))
    with_mask = bool(np.any(msk))

    ln1_w = np.asarray(ln1_w, np.float32); ln1_b = np.asarray(ln1_b, np.float32)
    ln2_w = np.asarray(ln2_w, np.float32); ln2_b = np.asarray(ln2_b, np.float32)
    q_w = np.asarray(q_w, np.float32); q_b = np.asarray(q_b, np.float32)
    k_w = np.asarray(k_w, np.float32); k_b = np.asarray(k_b, np.float32)
    v_w = np.asarray(v_w, np.float32); v_b = np.asarray(v_b, np.float32)
    o_w = np.asarray(o_w, np.float32); o_b = np.asarray(o_b, np.float32)
    fc1_w = np.asarray(fc1_w, np.float32); fc1_b = np.asarray(fc1_b, np.float32)
    fc2_w = np.asarray(fc2_w, np.float32); fc2_b = np.asarray(fc2_b, np.float32)

    scale = D ** -0.5
    # fold LN1 scale/bias into Q/K/V, and the softmax scale into Q
    qw_eff = (q_w * ln1_w[None, :]) * scale
    qb_eff = (q_b + q_w @ ln1_b) * scale
    kw_eff = k_w * ln1_w[None, :]
    kb_eff = k_b + k_w @ ln1_b
    vw_eff = v_w * ln1_w[None, :]
    vb_eff = v_b + v_w @ ln1_b
    # fold LN2 into fc1
    f1w_eff = fc1_w * ln2_w[None, :]
    f1b_eff = fc1_b + fc1_w @ ln2_b

    base = {
        "qw": _pack_lhsT(qw_eff),
        "kw": _pack_lhsT(kw_eff),
        "vw": np.ascontiguousarray(
            vw_eff.T.reshape(KC, 128, E)).astype(ml_dtypes.bfloat16),
        "ow": _pack_lhsT(o_w),
        "f1w": _pack_lhsT(f1w_eff),
        "f2w": _pack_lhsT(fc2_w),
        "qb": _pack_pbias(qb_eff),
        "kb": _pack_pbias(kb_eff),
        "vb": np.ascontiguousarray(vb_eff[None, :].astype(np.float32)),
        "ob": _pack_pbias(o_b),
        "f1b": _pack_pbias(f1b_eff),
        "f2b": _pack_pbias(fc2_b),
    }

    with_vbias = bool(np.any(vb_eff))
    key = (with_mask, with_vbias)
    if key not in _cache:
        _cache[key] = _build(with_mask, with_vbias)
    nc = _cache[key]

    in_maps = []
    for c in range(N_CORES):
        x = hs[c * B_LOC:(c + 1) * B_LOC].reshape(NT, E).T
        m = dict(base)
        m["xT"] = np.ascontiguousarray(x)
        m["xTb"] = np.ascontiguousarray(x).astype(ml_dtypes.bfloat16)
        if with_mask:
            m["mskT"] = np.ascontiguousarray(
                msk[c * B_LOC:(c + 1) * B_LOC, 0].transpose(0, 2, 1))
        in_maps.append(m)

    res = run_bass_kernel_spmd(nc, in_maps, core_ids=list(range(N_CORES)),
                               trace=TRACE)
    LAST_EXEC_NS = res.exec_time_ns

    outs = []
    for c in range(N_CORES):
        oT = res.results[c]["outT"]          # [E, NT] f32
        outs.append(np.ascontiguousarray(oT.T).reshape(B_LOC, S, E))
    return np.concatenate(outs, axis=0)


# revision 24
# speedup vs baseline: 1.0744x; 1.0012x over previous
"""Trainium2 Bass kernel for a CLIP encoder layer (B=32, S=257, E=1024, H=16, I=4096).

Strategy: data-parallel over batch across 8 NeuronCores (4 batch elements per
core), no collectives.  Per-core compute is done feature-major ([E, tokens])
so projection matmuls need no on-device transposes:

  - LayerNorm: column stats via PE ones-matmuls (f32r), normalization applied
    with two DVE passes; LN scale/bias are folded into the projection weights
    on the host.
  - Q/K/O/fc1/fc2: weight-stationary matmuls (lhsT = W^T packed on host,
    bf16), fp32 PSUM accumulation, N=257 (one batch element) moving slices.
  - V: activation-stationary -> token-major [tok, H, 65] with a ones column,
    so the softmax denominators fall out of the ctx matmul for free.
  - Attention: scores computed transposed (scores^T[j, i]) so softmax reduces
    over the partition dim via the ctx matmul; exp on ACT straight from PSUM;
    two heads (D=64) packed per PE pass at partition bases 0/64.
"""

import numpy as np
import ml_dtypes

B, S, E, H, D, II = 32, 257, 1024, 16, 64, 4096
N_CORES = 8
B_LOC = B // N_CORES          # 4
NT = B_LOC * S                # 1028
KC = E // 128                 # 8
MC_E = E // 128               # 8
MC_I = II // 128              # 32
EPS = 1e-5

# token slices within NT, used for f32r matmuls (N>=256 except the 4-tail)
LN_SLICES = [(0, 512), (512, 1024), (1024, NT)]
# j-chunks of one batch element's 257 keys
JC = [(0, 128), (128, 128), (256, 1)]

TRACE = False
LAST_EXEC_NS = None

_cache = {}


def _build(with_mask: bool, with_vbias: bool, with_qkbias: bool):
    import concourse.tile as tile
    from concourse import bacc, mybir
    from contextlib import ExitStack

    F32 = mybir.dt.float32
    BF16 = mybir.dt.bfloat16
    AF = mybir.ActivationFunctionType
    ALU = mybir.AluOpType

    nc = bacc.Bacc("TRN2", target_bir_lowering=False, debug=False,
                   enable_asserts=False, num_devices=N_CORES)

    xT_d = nc.dram_tensor("xT", [E, NT], F32, kind="ExternalInput")
    xTb_d = nc.dram_tensor("xTb", [E, NT], BF16, kind="ExternalInput")
    qw_d = nc.dram_tensor("qw", [MC_E, 128, KC, 128], BF16, kind="ExternalInput")
    kw_d = nc.dram_tensor("kw", [MC_E, 128, KC, 128], BF16, kind="ExternalInput")
    vw_d = nc.dram_tensor("vw", [KC, 128, E], BF16, kind="ExternalInput")
    ow_d = nc.dram_tensor("ow", [MC_E, 128, KC, 128], BF16, kind="ExternalInput")
    f1w_d = nc.dram_tensor("f1w", [MC_I, 128, KC, 128], BF16, kind="ExternalInput")
    f2w_d = nc.dram_tensor("f2w", [MC_E, 128, MC_I, 128], BF16, kind="ExternalInput")
    qb_d = nc.dram_tensor("qb", [128, MC_E], F32, kind="ExternalInput")
    kb_d = nc.dram_tensor("kb", [128, MC_E], F32, kind="ExternalInput")
    vb_d = nc.dram_tensor("vb", [1, E], F32, kind="ExternalInput")
    ob_d = nc.dram_tensor("ob", [128, MC_E], F32, kind="ExternalInput")
    f1b_d = nc.dram_tensor("f1b", [128, MC_I], F32, kind="ExternalInput")
    f2b_d = nc.dram_tensor("f2b", [128, MC_E], F32, kind="ExternalInput")
    mskT_d = None
    if with_mask:
        mskT_d = nc.dram_tensor("mskT", [B_LOC, S, S], F32, kind="ExternalInput")
    outT_d = nc.dram_tensor("outT", [E, NT], F32, kind="ExternalOutput")

    with tile.TileContext(nc) as tc, ExitStack() as top:
        consts = top.enter_context(tc.tile_pool(name="consts", bufs=1))

        ones_col = consts.tile([128, 1], BF16)
        nc.vector.memset(ones_col[:], 1.0)
        ones_row = consts.tile([1, 128], BF16)
        nc.vector.memset(ones_row[:], 1.0)
        eps_t = consts.tile([1, 1], F32)
        nc.vector.memset(eps_t[:], EPS)
        qb_sb = consts.tile([128, MC_E], F32)
        nc.sync.dma_start(out=qb_sb[:], in_=qb_d[:])
        kb_sb = consts.tile([128, MC_E], F32)
        nc.sync.dma_start(out=kb_sb[:], in_=kb_d[:])
        ob_sb = consts.tile([128, MC_E], F32)
        nc.sync.dma_start(out=ob_sb[:], in_=ob_d[:])
        f2b_sb = consts.tile([128, MC_E], F32)
        nc.sync.dma_start(out=f2b_sb[:], in_=f2b_d[:])
        f1b_sb = consts.tile([128, MC_I], F32)
        nc.sync.dma_start(out=f1b_sb[:], in_=f1b_d[:])
        vb_sb = consts.tile([128, E], F32)
        nc.sync.dma_start(out=vb_sb[:], in_=vb_d[0:1, :].to_broadcast((128, E)))

        def emit_ln(ph, src_ap, srcbf_ap, out_pool, sfx):
            """Per-batch column LayerNorm over the feature (partition) dim.
            src_ap(k, b) -> [128, S] f32 AP; srcbf_ap(k, b) -> [128, S] bf16
            AP.  Returns {(k, b): [128, S] bf16 tile} of (x - mu) * rstd
            (LN scale/bias folded into downstream weights host-side)."""
            lntmp = ph.enter_context(tc.tile_pool(name=f"lntmp{sfx}", bufs=3))
            sqp = ph.enter_context(tc.tile_pool(name=f"sqp{sfx}", bufs=3))
            rows = ph.enter_context(tc.tile_pool(name=f"rows{sfx}", bufs=8))
            pstat = ph.enter_context(
                tc.tile_pool(name=f"pstat{sfx}", bufs=2, space="PSUM"))
            pbc = ph.enter_context(
                tc.tile_pool(name=f"pbc{sfx}", bufs=2, space="PSUM"))
            outs = {}
            for b in range(B_LOC):
                ps_sum = pstat.tile([1, 512], F32, name="ps_sum", tag="stat")
                ps_sq = pstat.tile([1, 512], F32, name="ps_sq", tag="stat")
                for k in range(KC):
                    xb = srcbf_ap(k, b)
                    sq = sqp.tile([128, S], BF16, name="sq", tag="sq")
                    nc.scalar.activation(out=sq[:], in_=xb, func=AF.Square)
                    nc.tensor.matmul(ps_sum[0:1, 0:S], ones_col[:], xb,
                                     start=(k == 0), stop=(k == KC - 1))
                    nc.tensor.matmul(ps_sq[0:1, 0:S], ones_col[:], sq[:],
                                     start=(k == 0), stop=(k == KC - 1))
                musq = rows.tile([1, S], F32, name="musq", tag="row")
                nc.scalar.activation(out=musq[0:1, :], in_=ps_sum[0:1, 0:S],
                                     func=AF.Square, scale=-1.0 / E)
                muneg_b = rows.tile([1, S], BF16, name="muneg_b", tag="row")
                nc.scalar.mul(out=muneg_b[0:1, :], in_=ps_sum[0:1, 0:S],
                              mul=-1.0 / E)
                var = rows.tile([1, S], F32, name="var", tag="row")
                nc.vector.scalar_tensor_tensor(
                    out=var[0:1, :], in0=ps_sq[0:1, 0:S], scalar=1.0 / E,
                    in1=musq[0:1, :], op0=ALU.mult, op1=ALU.subtract)
                sd = rows.tile([1, S], F32, name="sd", tag="row")
                nc.scalar.activation(out=sd[0:1, :], in_=var[0:1, :],
                                     func=AF.Sqrt, bias=eps_t[0:1, 0:1])
                rstd = rows.tile([1, S], F32, name="rstd", tag="row")
                nc.vector.reciprocal_approx_fast(out=rstd[0:1, :],
                                                 in_=sd[0:1, :])
                rstd_b = rows.tile([1, S], BF16, name="rstd_b", tag="row")
                nc.vector.tensor_copy(out=rstd_b[0:1, :], in_=rstd[0:1, :])
                psA = pbc.tile([128, 512], F32, name="psA", tag="bc")
                psB = pbc.tile([128, 512], F32, name="psB", tag="bc")
                nc.tensor.matmul(psA[:, 0:S], ones_row[0:1, :],
                                 rstd_b[0:1, :], start=True, stop=True)
                nc.tensor.matmul(psB[:, 0:S], ones_row[0:1, :],
                                 muneg_b[0:1, :], start=True, stop=True)
                for k in range(KC):
                    tmp = lntmp.tile([128, S], F32, name="tmp", tag="ap")
                    nc.vector.tensor_add(out=tmp[:], in0=src_ap(k, b),
                                         in1=psB[:, 0:S])
                    o = out_pool.tile([128, S], BF16, name="lno", tag="lno")
                    nc.vector.tensor_mul(out=o[:], in0=tmp[:], in1=psA[:, 0:S])
                    outs[(k, b)] = o
            return outs

        with tc.tile_pool(name="xt", bufs=KC * B_LOC) as xt_p:
            with tc.tile_pool(name="ctxT", bufs=MC_E) as ctx_p:
                ctxT = [ctx_p.tile([128, NT], BF16, tag="ctxT", name="ctxT")
                        for _ in range(MC_E)]

                # ============= LN1 / V / QK+attention ====================
                with tc.tile_pool(name="xln1", bufs=KC * B_LOC) as xln1_p:
                    with ExitStack() as ln1_ph:
                        xtb_p = ln1_ph.enter_context(
                            tc.tile_pool(name="xtb", bufs=KC * B_LOC))
                        xtb = {}
                        for b in range(B_LOC):
                            for k in range(KC):
                                tb = xtb_p.tile([128, S], BF16, name="xtb",
                                                tag="xtb")
                                nc.sync.dma_start(
                                    out=tb[:],
                                    in_=xTb_d[k * 128:(k + 1) * 128,
                                              b * S:(b + 1) * S])
                                xtb[(k, b)] = tb
                        xln1 = emit_ln(ln1_ph,
                                       lambda k, b: xtb[(k, b)][:],
                                       lambda k, b: xtb[(k, b)][:],
                                       xln1_p, "1")

                        # ============= V projection ======================
                        v_p = ln1_ph.enter_context(
                            tc.tile_pool(name="vpool", bufs=3 * B_LOC))
                        v_tiles = {}
                        with ExitStack() as ph:
                            vw_p = ph.enter_context(
                                tc.tile_pool(name="vw", bufs=1))
                            ppv = ph.enter_context(
                                tc.tile_pool(name="ppv", bufs=2, space="PSUM"))
                            vw_sb = vw_p.tile([128, KC, E], BF16)
                            for k in range(KC):
                                nc.sync.dma_start(out=vw_sb[:, k, :],
                                                  in_=vw_d[k, :, :])
                            for b in range(B_LOC):
                                for jc, (j0, jcs) in enumerate(JC):
                                    ps = ppv.tile([128, 2, 512], F32,
                                                  name="vps", tag="vps")
                                    for n in range(2):
                                        for k in range(KC):
                                            nc.tensor.matmul(
                                                ps[0:jcs, n, :],
                                                xln1[(k, b)][:, j0:j0 + jcs],
                                                vw_sb[:, k,
                                                      n * 512:(n + 1) * 512],
                                                start=(k == 0),
                                                stop=(k == KC - 1))
                                    # [tok, H, 128]: cols 0:64 ones, cols
                                    # 64:128 V -> ctx matmul replicates the
                                    # softmax sums across partitions 0:64
                                    # (base 0: custom-DVE recip needs it).
                                    vt = v_p.tile([128, H, 128], BF16,
                                                  name="vt", tag="vt")
                                    if with_vbias:
                                        nc.vector.tensor_add(
                                            out=vt[0:jcs, :, 64:128],
                                            in0=ps[0:jcs, :, :],
                                            in1=vb_sb[0:jcs, :])
                                    else:
                                        nc.scalar.copy(
                                            out=vt[0:jcs, :, 64:128],
                                            in_=ps[0:jcs, :, :])
                                    nc.gpsimd.memset(vt[:, :, 0:64], 1.0)
                                    v_tiles[(b, jc)] = vt

                    # ========= Q/K + attention (per head-pair chunk) =====
                    with ExitStack() as ph:
                        qt_p = ph.enter_context(tc.tile_pool(name="qt", bufs=2))
                        kt_p = ph.enter_context(tc.tile_pool(name="kt", bufs=2))
                        wqk_p = ph.enter_context(
                            tc.tile_pool(name="wqk", bufs=3))
                        e_p = ph.enter_context(tc.tile_pool(name="ep", bufs=6))
                        rs_p = ph.enter_context(tc.tile_pool(name="rsp", bufs=3))
                        if with_mask:
                            msk_p = ph.enter_context(
                                tc.tile_pool(name="mskp", bufs=3 * B_LOC))
                        pp2 = ph.enter_context(
                            tc.tile_pool(name="pp2", bufs=1, space="PSUM"))
                        psp = ph.enter_context(
                            tc.tile_pool(name="psp", bufs=3, space="PSUM"))
                        if with_mask:
                            msk = {}
                            for b in range(B_LOC):
                                for jc, (j0, jcs) in enumerate(JC):
                                    mt = msk_p.tile([128, S], F32, name="mt",
                                                    tag="mt")
                                    nc.sync.dma_start(
                                        out=mt[0:jcs, :],
                                        in_=mskT_d[b, j0:j0 + jcs, :])
                                    msk[(b, jc)] = mt

                        for ec in range(MC_E):
                            qkt = []
                            for (w_d, b_sb, opool) in (
                                    (qw_d, qb_sb, qt_p),
                                    (kw_d, kb_sb, kt_p)):
                                wt = wqk_p.tile([128, KC, 128], BF16,
                                                name="wqk", tag="wqk")
                                nc.sync.dma_start(out=wt[:],
                                                  in_=w_d[ec, :, :, :])
                                ot = opool.tile([128, NT], BF16,
                                                name="qk", tag="qk")
                                for half in range(2):
                                    ps = pp2.tile([128, 2, 512], F32,
                                                  name="pqk", tag="pqk")
                                    for bb in range(2):
                                        b = half * 2 + bb
                                        for k in range(KC):
                                            nc.tensor.matmul(
                                                ps[:, bb, 0:S],
                                                wt[:, k, :],
                                                xln1[(k, b)][:],
                                                start=(k == 0),
                                                stop=(k == KC - 1))
                                    if with_qkbias:
                                        nc.vector.tensor_scalar_add(
                                            out=ot[:, half * 2 * S:
                                                   (half + 1) * 2 * S],
                                            in0=ps[:, :, 0:S],
                                            scalar1=b_sb[:, ec:ec + 1])
                                    else:
                                        nc.scalar.copy(
                                            out=ot[:, half * 2 * S:
                                                   (half + 1) * 2 * S],
                                            in_=ps[:, :, 0:S])
                                qkt.append(ot)
                            qte, kte = qkt

                            for b in range(B_LOC):
                                ets = []
                                for jc, (j0, jcs) in enumerate(JC):
                                    sp = psp.tile([128, 2, 512], F32,
                                                  name="sp", tag="sp")
                                    for hi in range(2):
                                        p0 = hi * 64
                                        nc.tensor.matmul(
                                            sp[0:jcs, hi, 0:S],
                                            kte[p0:p0 + 64,
                                                b * S + j0: b * S + j0 + jcs],
                                            qte[p0:p0 + 64,
                                                b * S:(b + 1) * S],
                                            start=True, stop=True)
                                    if with_mask:
                                        for hi in range(2):
                                            nc.vector.tensor_add(
                                                out=sp[0:jcs, hi, 0:S],
                                                in0=sp[0:jcs, hi, 0:S],
                                                in1=msk[(b, jc)][0:jcs, :])
                                    et = e_p.tile([128, 2, S], BF16,
                                                  name="et", tag="et")
                                    nc.scalar.activation(
                                        out=et[0:jcs, :, :],
                                        in_=sp[0:jcs, :, 0:S], func=AF.Exp)
                                    ets.append(et)
                                cp = psp.tile([128, 2, 512], F32,
                                              name="cp", tag="sp")
                                for hi in range(2):
                                    h = 2 * ec + hi
                                    for jc, (j0, jcs) in enumerate(JC):
                                        nc.tensor.matmul(
                                            cp[0:128, hi, 0:S],
                                            v_tiles[(b, jc)][0:jcs, h, :],
                                            ets[jc][0:jcs, hi, :],
                                            start=(jc == 0), stop=(jc == 2))
                                rst = rs_p.tile([64, 2, S], F32,
                                                name="rst", tag="rst")
                                nc.vector.reciprocal_approx_fast(
                                    out=rst[0:64, :, :],
                                    in_=cp[0:64, :, 0:S])
                                for hi in range(2):
                                    nc.vector.tensor_mul(
                                        out=ctxT[ec][hi * 64:hi * 64 + 64,
                                                     b * S:(b + 1) * S],
                                        in0=cp[64:128, hi, 0:S],
                                        in1=rst[0:64, hi, :])

                # residual x^T (f32) loads late - DMA is idle by now
                xt = {}
                for b in range(B_LOC):
                    for k in range(KC):
                        t = xt_p.tile([128, S], F32, name="xt", tag="xt")
                        nc.sync.dma_start(
                            out=t[:],
                            in_=xT_d[k * 128:(k + 1) * 128,
                                     b * S:(b + 1) * S])
                        xt[(k, b)] = t

                # xln1 closed; right-side long-lived pools
                ht_p = top.enter_context(
                    tc.tile_pool(name="ht", bufs=KC, side="right"))
                f1o_p = top.enter_context(
                    tc.tile_pool(name="f1o", bufs=MC_I, side="right"))

                # ============= out projection + residual =================
                ht = []
                with ExitStack() as ph:
                    wo_p = ph.enter_context(tc.tile_pool(name="wo", bufs=3))
                    ppo = ph.enter_context(
                        tc.tile_pool(name="ppo", bufs=2, space="PSUM"))
                    for m in range(MC_E):
                        wt = wo_p.tile([128, KC, 128], BF16, name="wo",
                                       tag="wo")
                        nc.sync.dma_start(out=wt[:], in_=ow_d[m, :, :, :])
                        ps = ppo.tile([128, B_LOC, 512], F32, name="po",
                                      tag="po")
                        for b in range(B_LOC):
                            for k in range(KC):
                                nc.tensor.matmul(
                                    ps[:, b, 0:S], wt[:, k, :],
                                    ctxT[k][:, b * S:(b + 1) * S],
                                    start=(k == 0), stop=(k == KC - 1))
                        o = ht_p.tile([128, NT], F32, name="ht", tag="ht")
                        for b in range(B_LOC):
                            nc.vector.scalar_tensor_tensor(
                                out=o[:, b * S:(b + 1) * S],
                                in0=ps[:, b, 0:S],
                                scalar=ob_sb[:, m:m + 1],
                                in1=xt[(m, b)][:],
                                op0=ALU.add, op1=ALU.add)
                        ht.append(o)
            # ctxT closed
        # xt closed

        # ================= LN2 + MLP =====================================
        with tc.tile_pool(name="xln2", bufs=KC * B_LOC) as xln2_p:
            f1o = []
            with ExitStack() as ln2_ph:
                htb_p = ln2_ph.enter_context(tc.tile_pool(name="htb", bufs=KC))
                htb = []
                for k in range(KC):
                    hb = htb_p.tile([128, NT], BF16, name="htb", tag="htb")
                    nc.vector.tensor_copy(out=hb[:], in_=ht[k][:])
                    htb.append(hb)
                xln2 = emit_ln(
                    ln2_ph,
                    lambda k, b: ht[k][:, b * S:(b + 1) * S],
                    lambda k, b: htb[k][:, b * S:(b + 1) * S],
                    xln2_p, "2")
                wf1_p = ln2_ph.enter_context(tc.tile_pool(name="wf1", bufs=3))
                ppf1 = ln2_ph.enter_context(
                    tc.tile_pool(name="ppf1", bufs=2, space="PSUM"))
                for m in range(MC_I):
                    wt = wf1_p.tile([128, KC, 128], BF16, name="wf1",
                                    tag="wf1")
                    nc.sync.dma_start(out=wt[:], in_=f1w_d[m, :, :, :])
                    o = f1o_p.tile([128, NT], BF16, name="f1o", tag="f1o")
                    for half in range(2):
                        ps = ppf1.tile([128, 2, 512], F32, name="pf1",
                                       tag="pf1")
                        for bb in range(2):
                            b = half * 2 + bb
                            for k in range(KC):
                                nc.tensor.matmul(
                                    ps[:, bb, 0:S], wt[:, k, :],
                                    xln2[(k, b)][:],
                                    start=(k == 0), stop=(k == KC - 1))
                        nc.scalar.activation(
                            out=o[:, half * 2 * S:(half + 1) * 2 * S],
                            in_=ps[:, :, 0:S],
                            func=AF.Gelu_apprx_tanh,
                            bias=f1b_sb[:, m:m + 1])
                    f1o.append(o)

        with ExitStack() as ph:
            wf2_p = ph.enter_context(tc.tile_pool(name="wf2", bufs=2))
            ppf2 = ph.enter_context(
                tc.tile_pool(name="ppf2", bufs=2, space="PSUM"))
            out_p = ph.enter_context(tc.tile_pool(name="outp", bufs=3))
            for m in range(MC_E):
                wt = wf2_p.tile([128, MC_I, 128], BF16, name="wf2", tag="wf2")
                nc.sync.dma_start(out=wt[:], in_=f2w_d[m, :, :, :])
                ps = ppf2.tile([128, B_LOC, 512], F32, name="pf2", tag="pf2")
                for b in range(B_LOC):
                    for k in range(MC_I):
                        nc.tensor.matmul(
                            ps[:, b, 0:S], wt[:, k, :],
                            f1o[k][:, b * S:(b + 1) * S],
                            start=(k == 0), stop=(k == MC_I - 1))
                o = out_p.tile([128, NT], F32, name="oo", tag="oo")
                nc.vector.scalar_tensor_tensor(
                    out=o[:], in0=ps[:, :, 0:S], scalar=f2b_sb[:, m:m + 1],
                    in1=ht[m][:], op0=ALU.add, op1=ALU.add)
                nc.sync.dma_start(out=outT_d[m * 128:(m + 1) * 128, :],
                                  in_=o[:])

    nc.compile()
    return nc


def _pack_lhsT(W):
    """W [M, K] (out, in) -> [M/128, 128, K/128, 128] bf16 with
    [m, p, k, j] = W[m*128+j, k*128+p] (lhsT tiles, partition = K)."""
    W = np.asarray(W, np.float32)
    M, K = W.shape
    A = W.reshape(M // 128, 128, K // 128, 128)
    return np.ascontiguousarray(A.transpose(0, 3, 2, 1)).astype(ml_dtypes.bfloat16)


def _pack_pbias(b):
    """b [M] -> [128, M/128] f32 per-partition bias columns."""
    return np.ascontiguousarray(np.asarray(b, np.float32).reshape(-1, 128).T)


def kernel(hidden_states, attention_mask, causal_attention_mask,
           ln1_w, ln1_b, q_w, q_b, k_w, k_b, v_w, v_b, o_w, o_b,
           ln2_w, ln2_b, fc1_w, fc1_b, fc2_w, fc2_b):
    global LAST_EXEC_NS
    from concourse.bass_utils import run_bass_kernel_spmd

    hs = np.asarray(hidden_states, np.float32)
    msk = (np.asarray(attention_mask, np.float32)
           + np.asarray(causal_attention_mask, np.float32))
    with_mask = bool(np.any(msk))

    ln1_w = np.asarray(ln1_w, np.float32); ln1_b = np.asarray(ln1_b, np.float32)
    ln2_w = np.asarray(ln2_w, np.float32); ln2_b = np.asarray(ln2_b, np.float32)
    q_w = np.asarray(q_w, np.float32); q_b = np.asarray(q_b, np.float32)
    k_w = np.asarray(k_w, np.float32); k_b = np.asarray(k_b, np.float32)
    v_w = np.asarray(v_w, np.float32); v_b = np.asarray(v_b, np.float32)
    o_w = np.asarray(o_w, np.float32); o_b = np.asarray(o_b, np.float32)
    fc1_w = np.asarray(fc1_w, np.float32); fc1_b = np.asarray(fc1_b, np.float32)
    fc2_w = np.asarray(fc2_w, np.float32); fc2_b = np.asarray(fc2_b, np.float32)

    scale = D ** -0.5
    # fold LN1 scale/bias into Q/K/V, and the softmax scale into Q
    qw_eff = (q_w * ln1_w[None, :]) * scale
    qb_eff = (q_b + q_w @ ln1_b) * scale
    kw_eff = k_w * ln1_w[None, :]
    kb_eff = k_b + k_w @ ln1_b
    vw_eff = v_w * ln1_w[None, :]
    vb_eff = v_b + v_w @ ln1_b
    # fold LN2 into fc1
    f1w_eff = fc1_w * ln2_w[None, :]
    f1b_eff = fc1_b + fc1_w @ ln2_b

    base = {
        "qw": _pack_lhsT(qw_eff),
        "kw": _pack_lhsT(kw_eff),
        "vw": np.ascontiguousarray(
            vw_eff.T.reshape(KC, 128, E)).astype(ml_dtypes.bfloat16),
        "ow": _pack_lhsT(o_w),
        "f1w": _pack_lhsT(f1w_eff),
        "f2w": _pack_lhsT(fc2_w),
        "qb": _pack_pbias(qb_eff),
        "kb": _pack_pbias(kb_eff),
        "vb": np.ascontiguousarray(vb_eff[None, :].astype(np.float32)),
        "ob": _pack_pbias(o_b),
        "f1b": _pack_pbias(f1b_eff),
        "f2b": _pack_pbias(fc2_b),
    }

    with_vbias = bool(np.any(vb_eff))
    with_qkbias = bool(np.any(qb_eff)) or bool(np.any(kb_eff))
    key = (with_mask, with_vbias, with_qkbias)
    if key not in _cache:
        _cache[key] = _build(with_mask, with_vbias, with_qkbias)
    nc = _cache[key]

    in_maps = []
    for c in range(N_CORES):
        x = hs[c * B_LOC:(c + 1) * B_LOC].reshape(NT, E).T
        m = dict(base)
        m["xT"] = np.ascontiguousarray(x)
        m["xTb"] = np.ascontiguousarray(x).astype(ml_dtypes.bfloat16)
        if with_mask:
            m["mskT"] = np.ascontiguousarray(
                msk[c * B_LOC:(c + 1) * B_LOC, 0].transpose(0, 2, 1))
        in_maps.append(m)

    res = run_bass_kernel_spmd(nc, in_maps, core_ids=list(range(N_CORES)),
                               trace=TRACE)
    LAST_EXEC_NS = res.exec_time_ns

    outs = []
    for c in range(N_CORES):
        oT = res.results[c]["outT"]          # [E, NT] f32
        outs.append(np.ascontiguousarray(oT.T).reshape(B_LOC, S, E))
    return np.concatenate(outs, axis=0)


# revision 25
# speedup vs baseline: 1.0831x; 1.0081x over previous
"""Trainium2 Bass kernel for a CLIP encoder layer (B=32, S=257, E=1024, H=16, I=4096).

Strategy: data-parallel over batch across 8 NeuronCores (4 batch elements per
core), no collectives.  Per-core compute is done feature-major ([E, tokens])
so projection matmuls need no on-device transposes:

  - LayerNorm: column stats via PE ones-matmuls (f32r), normalization applied
    with two DVE passes; LN scale/bias are folded into the projection weights
    on the host.
  - Q/K/O/fc1/fc2: weight-stationary matmuls (lhsT = W^T packed on host,
    bf16), fp32 PSUM accumulation, N=257 (one batch element) moving slices.
  - V: activation-stationary -> token-major [tok, H, 65] with a ones column,
    so the softmax denominators fall out of the ctx matmul for free.
  - Attention: scores computed transposed (scores^T[j, i]) so softmax reduces
    over the partition dim via the ctx matmul; exp on ACT straight from PSUM;
    two heads (D=64) packed per PE pass at partition bases 0/64.
"""

import numpy as np
import ml_dtypes

B, S, E, H, D, II = 32, 257, 1024, 16, 64, 4096
N_CORES = 8
B_LOC = B // N_CORES          # 4
NT = B_LOC * S                # 1028
KC = E // 128                 # 8
MC_E = E // 128               # 8
MC_I = II // 128              # 32
EPS = 1e-5

# token slices within NT, used for f32r matmuls (N>=256 except the 4-tail)
LN_SLICES = [(0, 512), (512, 1024), (1024, NT)]
# j-chunks of one batch element's 257 keys
JC = [(0, 128), (128, 128), (256, 1)]

TRACE = False
LAST_EXEC_NS = None

_cache = {}


def _build(with_mask: bool, with_vbias: bool, with_qkbias: bool):
    import concourse.tile as tile
    from concourse import bacc, mybir
    from contextlib import ExitStack

    F32 = mybir.dt.float32
    BF16 = mybir.dt.bfloat16
    AF = mybir.ActivationFunctionType
    ALU = mybir.AluOpType

    nc = bacc.Bacc("TRN2", target_bir_lowering=False, debug=False,
                   enable_asserts=False, num_devices=N_CORES)

    xT_d = nc.dram_tensor("xT", [E, NT], F32, kind="ExternalInput")
    xTb_d = nc.dram_tensor("xTb", [E, NT], BF16, kind="ExternalInput")
    qw_d = nc.dram_tensor("qw", [MC_E, 128, KC, 128], BF16, kind="ExternalInput")
    kw_d = nc.dram_tensor("kw", [MC_E, 128, KC, 128], BF16, kind="ExternalInput")
    vw_d = nc.dram_tensor("vw", [KC, 128, E], BF16, kind="ExternalInput")
    ow_d = nc.dram_tensor("ow", [MC_E, 128, KC, 128], BF16, kind="ExternalInput")
    f1w_d = nc.dram_tensor("f1w", [MC_I, 128, KC, 128], BF16, kind="ExternalInput")
    f2w_d = nc.dram_tensor("f2w", [MC_E, 128, MC_I, 128], BF16, kind="ExternalInput")
    qb_d = nc.dram_tensor("qb", [128, MC_E], F32, kind="ExternalInput")
    kb_d = nc.dram_tensor("kb", [128, MC_E], F32, kind="ExternalInput")
    vb_d = nc.dram_tensor("vb", [1, E], F32, kind="ExternalInput")
    ob_d = nc.dram_tensor("ob", [128, MC_E], F32, kind="ExternalInput")
    f1b_d = nc.dram_tensor("f1b", [128, MC_I], F32, kind="ExternalInput")
    f2b_d = nc.dram_tensor("f2b", [128, MC_E], F32, kind="ExternalInput")
    mskT_d = None
    if with_mask:
        mskT_d = nc.dram_tensor("mskT", [B_LOC, S, S], F32, kind="ExternalInput")
    outT_d = nc.dram_tensor("outT", [E, NT], F32, kind="ExternalOutput")

    with tile.TileContext(nc) as tc, ExitStack() as top:
        consts = top.enter_context(tc.tile_pool(name="consts", bufs=1))

        ones_col = consts.tile([128, 1], BF16)
        nc.vector.memset(ones_col[:], 1.0)
        ones_row = consts.tile([1, 128], BF16)
        nc.vector.memset(ones_row[:], 1.0)
        eps_t = consts.tile([1, 1], F32)
        nc.vector.memset(eps_t[:], EPS)
        qb_sb = consts.tile([128, MC_E], F32)
        nc.sync.dma_start(out=qb_sb[:], in_=qb_d[:])
        kb_sb = consts.tile([128, MC_E], F32)
        nc.sync.dma_start(out=kb_sb[:], in_=kb_d[:])
        ob_sb = consts.tile([128, MC_E], F32)
        nc.sync.dma_start(out=ob_sb[:], in_=ob_d[:])
        f2b_sb = consts.tile([128, MC_E], F32)
        nc.sync.dma_start(out=f2b_sb[:], in_=f2b_d[:])
        f1b_sb = consts.tile([128, MC_I], F32)
        nc.sync.dma_start(out=f1b_sb[:], in_=f1b_d[:])
        vb_sb = consts.tile([128, E], F32)
        nc.sync.dma_start(out=vb_sb[:], in_=vb_d[0:1, :].to_broadcast((128, E)))

        def emit_ln(ph, src_ap, srcbf_ap, out_pool, sfx):
            """Per-batch column LayerNorm over the feature (partition) dim.
            src_ap(k, b) -> [128, S] f32 AP; srcbf_ap(k, b) -> [128, S] bf16
            AP.  Returns {(k, b): [128, S] bf16 tile} of (x - mu) * rstd
            (LN scale/bias folded into downstream weights host-side)."""
            lntmp = ph.enter_context(tc.tile_pool(name=f"lntmp{sfx}", bufs=3))
            sqp = ph.enter_context(tc.tile_pool(name=f"sqp{sfx}", bufs=3))
            rows = ph.enter_context(tc.tile_pool(name=f"rows{sfx}", bufs=8))
            pstat = ph.enter_context(
                tc.tile_pool(name=f"pstat{sfx}", bufs=2, space="PSUM"))
            pbc = ph.enter_context(
                tc.tile_pool(name=f"pbc{sfx}", bufs=2, space="PSUM"))
            outs = {}
            for b in range(B_LOC):
                ps_sum = pstat.tile([1, 512], F32, name="ps_sum", tag="stat")
                ps_sq = pstat.tile([1, 512], F32, name="ps_sq", tag="stat")
                for k in range(KC):
                    xb = srcbf_ap(k, b)
                    sq = sqp.tile([128, S], BF16, name="sq", tag="sq")
                    nc.scalar.activation(out=sq[:], in_=xb, func=AF.Square)
                    nc.tensor.matmul(ps_sum[0:1, 0:S], ones_col[:], xb,
                                     start=(k == 0), stop=(k == KC - 1))
                    nc.tensor.matmul(ps_sq[0:1, 0:S], ones_col[:], sq[:],
                                     start=(k == 0), stop=(k == KC - 1))
                musq = rows.tile([1, S], F32, name="musq", tag="row")
                nc.scalar.activation(out=musq[0:1, :], in_=ps_sum[0:1, 0:S],
                                     func=AF.Square, scale=-1.0 / E)
                muneg_b = rows.tile([1, S], BF16, name="muneg_b", tag="row")
                nc.scalar.mul(out=muneg_b[0:1, :], in_=ps_sum[0:1, 0:S],
                              mul=-1.0 / E)
                var = rows.tile([1, S], F32, name="var", tag="row")
                nc.vector.scalar_tensor_tensor(
                    out=var[0:1, :], in0=ps_sq[0:1, 0:S], scalar=1.0 / E,
                    in1=musq[0:1, :], op0=ALU.mult, op1=ALU.subtract)
                sd = rows.tile([1, S], F32, name="sd", tag="row")
                nc.scalar.activation(out=sd[0:1, :], in_=var[0:1, :],
                                     func=AF.Sqrt, bias=eps_t[0:1, 0:1])
                rstd = rows.tile([1, S], F32, name="rstd", tag="row")
                nc.vector.reciprocal_approx_fast(out=rstd[0:1, :],
                                                 in_=sd[0:1, :])
                rstd_b = rows.tile([1, S], BF16, name="rstd_b", tag="row")
                nc.vector.tensor_copy(out=rstd_b[0:1, :], in_=rstd[0:1, :])
                psA = pbc.tile([128, 512], F32, name="psA", tag="bc")
                psB = pbc.tile([128, 512], F32, name="psB", tag="bc")
                nc.tensor.matmul(psA[:, 0:S], ones_row[0:1, :],
                                 rstd_b[0:1, :], start=True, stop=True)
                nc.tensor.matmul(psB[:, 0:S], ones_row[0:1, :],
                                 muneg_b[0:1, :], start=True, stop=True)
                for k in range(KC):
                    tmp = lntmp.tile([128, S], F32, name="tmp", tag="ap")
                    nc.vector.tensor_add(out=tmp[:], in0=src_ap(k, b),
                                         in1=psB[:, 0:S])
                    o = out_pool.tile([128, S], BF16, name="lno", tag="lno")
                    nc.vector.tensor_mul(out=o[:], in0=tmp[:], in1=psA[:, 0:S])
                    outs[(k, b)] = o
            return outs

        with tc.tile_pool(name="xt", bufs=KC * B_LOC) as xt_p:
            with tc.tile_pool(name="ctxT", bufs=MC_E) as ctx_p:
                ctxT = [ctx_p.tile([128, NT], BF16, tag="ctxT", name="ctxT")
                        for _ in range(MC_E)]

                # ============= LN1 / V / QK+attention ====================
                with tc.tile_pool(name="xln1", bufs=KC * B_LOC) as xln1_p:
                    with ExitStack() as ln1_ph:
                        xtb_p = ln1_ph.enter_context(
                            tc.tile_pool(name="xtb", bufs=KC * B_LOC))
                        xtb = {}
                        for b in range(B_LOC):
                            for k in range(KC):
                                tb = xtb_p.tile([128, S], BF16, name="xtb",
                                                tag="xtb")
                                nc.sync.dma_start(
                                    out=tb[:],
                                    in_=xTb_d[k * 128:(k + 1) * 128,
                                              b * S:(b + 1) * S])
                                xtb[(k, b)] = tb
                        xln1 = emit_ln(ln1_ph,
                                       lambda k, b: xtb[(k, b)][:],
                                       lambda k, b: xtb[(k, b)][:],
                                       xln1_p, "1")

                        # ============= V projection ======================
                        v_p = ln1_ph.enter_context(
                            tc.tile_pool(name="vpool", bufs=3 * B_LOC))
                        v_tiles = {}
                        with ExitStack() as ph:
                            vw_p = ph.enter_context(
                                tc.tile_pool(name="vw", bufs=1))
                            ppv = ph.enter_context(
                                tc.tile_pool(name="ppv", bufs=2, space="PSUM"))
                            vw_sb = vw_p.tile([128, KC, E], BF16)
                            for k in range(KC):
                                nc.sync.dma_start(out=vw_sb[:, k, :],
                                                  in_=vw_d[k, :, :])
                            for b in range(B_LOC):
                                for jc, (j0, jcs) in enumerate(JC[:2]):
                                    ps = ppv.tile([128, 2, 512], F32,
                                                  name="vps", tag="vps")
                                    for n in range(2):
                                        for k in range(KC):
                                            nc.tensor.matmul(
                                                ps[0:jcs, n, :],
                                                xln1[(k, b)][:, j0:j0 + jcs],
                                                vw_sb[:, k,
                                                      n * 512:(n + 1) * 512],
                                                start=(k == 0),
                                                stop=(k == KC - 1))
                                    # [tok, H, 128]: cols 0:64 ones, cols
                                    # 64:128 V -> ctx matmul replicates the
                                    # softmax sums across partitions 0:64
                                    # (base 0: custom-DVE recip needs it).
                                    vt = v_p.tile([128, H, 128], BF16,
                                                  name="vt", tag="vt")
                                    if with_vbias:
                                        nc.vector.tensor_add(
                                            out=vt[0:jcs, :, 64:128],
                                            in0=ps[0:jcs, :, :],
                                            in1=vb_sb[0:jcs, :])
                                    else:
                                        nc.scalar.copy(
                                            out=vt[0:jcs, :, 64:128],
                                            in_=ps[0:jcs, :, :])
                                    nc.gpsimd.memset(vt[:, :, 0:64], 1.0)
                                    v_tiles[(b, jc)] = vt
                            # the 4 batches' tail token (j=256): pack the
                            # M=1 matmuls into column groups 0/32/64/96 so
                            # they run concurrently on the PE array.
                            ps = ppv.tile([128, 2, 512], F32,
                                          name="vps_t", tag="vps")
                            for b in range(B_LOC):
                                for n in range(2):
                                    for k in range(KC):
                                        nc.tensor.matmul(
                                            ps[32 * b:32 * b + 1, n, :],
                                            xln1[(k, b)][:, 256:257],
                                            vw_sb[:, k, n * 512:(n + 1) * 512],
                                            start=(k == 0), stop=(k == KC - 1),
                                            tile_position=(0, 32 * b))
                            for b in range(B_LOC):
                                vt = v_p.tile([128, H, 128], BF16,
                                              name="vt", tag="vt")
                                if with_vbias:
                                    nc.vector.tensor_add(
                                        out=vt[0:1, :, 64:128],
                                        in0=ps[32 * b:32 * b + 1, :, :],
                                        in1=vb_sb[0:1, :])
                                else:
                                    nc.scalar.copy(
                                        out=vt[0:1, :, 64:128],
                                        in_=ps[32 * b:32 * b + 1, :, :])
                                nc.gpsimd.memset(vt[:, :, 0:64], 1.0)
                                v_tiles[(b, 2)] = vt

                    # ========= Q/K + attention (per head-pair chunk) =====
                    with ExitStack() as ph:
                        qt_p = ph.enter_context(tc.tile_pool(name="qt", bufs=2))
                        kt_p = ph.enter_context(tc.tile_pool(name="kt", bufs=2))
                        wqk_p = ph.enter_context(
                            tc.tile_pool(name="wqk", bufs=4))
                        e_p = ph.enter_context(tc.tile_pool(name="ep", bufs=6))
                        rs_p = ph.enter_context(tc.tile_pool(name="rsp", bufs=3))
                        if with_mask:
                            msk_p = ph.enter_context(
                                tc.tile_pool(name="mskp", bufs=3 * B_LOC))
                        pp2 = ph.enter_context(
                            tc.tile_pool(name="pp2", bufs=1, space="PSUM"))
                        psp = ph.enter_context(
                            tc.tile_pool(name="psp", bufs=3, space="PSUM"))
                        if with_mask:
                            msk = {}
                            for b in range(B_LOC):
                                for jc, (j0, jcs) in enumerate(JC):
                                    mt = msk_p.tile([128, S], F32, name="mt",
                                                    tag="mt")
                                    nc.sync.dma_start(
                                        out=mt[0:jcs, :],
                                        in_=mskT_d[b, j0:j0 + jcs, :])
                                    msk[(b, jc)] = mt

                        for ec in range(MC_E):
                            qkt = []
                            for (w_d, b_sb, opool) in (
                                    (qw_d, qb_sb, qt_p),
                                    (kw_d, kb_sb, kt_p)):
                                wt = wqk_p.tile([128, KC, 128], BF16,
                                                name="wqk", tag="wqk")
                                nc.sync.dma_start(out=wt[:],
                                                  in_=w_d[ec, :, :, :])
                                ot = opool.tile([128, NT], BF16,
                                                name="qk", tag="qk")
                                for half in range(2):
                                    ps = pp2.tile([128, 2, 512], F32,
                                                  name="pqk", tag="pqk")
                                    for bb in range(2):
                                        b = half * 2 + bb
                                        for k in range(KC):
                                            nc.tensor.matmul(
                                                ps[:, bb, 0:S],
                                                wt[:, k, :],
                                                xln1[(k, b)][:],
                                                start=(k == 0),
                                                stop=(k == KC - 1))
                                    if with_qkbias:
                                        nc.vector.tensor_scalar_add(
                                            out=ot[:, half * 2 * S:
                                                   (half + 1) * 2 * S],
                                            in0=ps[:, :, 0:S],
                                            scalar1=b_sb[:, ec:ec + 1])
                                    else:
                                        nc.scalar.copy(
                                            out=ot[:, half * 2 * S:
                                                   (half + 1) * 2 * S],
                                            in_=ps[:, :, 0:S])
                                qkt.append(ot)
                            qte, kte = qkt

                            for b in range(B_LOC):
                                ets = []
                                for jc, (j0, jcs) in enumerate(JC):
                                    sp = psp.tile([128, 2, 512], F32,
                                                  name="sp", tag="sp")
                                    for hi in range(2):
                                        p0 = hi * 64
                                        nc.tensor.matmul(
                                            sp[0:jcs, hi, 0:S],
                                            kte[p0:p0 + 64,
                                                b * S + j0: b * S + j0 + jcs],
                                            qte[p0:p0 + 64,
                                                b * S:(b + 1) * S],
                                            start=True, stop=True)
                                    if with_mask:
                                        for hi in range(2):
                                            nc.vector.tensor_add(
                                                out=sp[0:jcs, hi, 0:S],
                                                in0=sp[0:jcs, hi, 0:S],
                                                in1=msk[(b, jc)][0:jcs, :])
                                    et = e_p.tile([128, 2, S], BF16,
                                                  name="et", tag="et")
                                    nc.scalar.activation(
                                        out=et[0:jcs, :, :],
                                        in_=sp[0:jcs, :, 0:S], func=AF.Exp)
                                    ets.append(et)
                                cp = psp.tile([128, 2, 512], F32,
                                              name="cp", tag="sp")
                                for hi in range(2):
                                    h = 2 * ec + hi
                                    for jc, (j0, jcs) in enumerate(JC):
                                        nc.tensor.matmul(
                                            cp[0:128, hi, 0:S],
                                            v_tiles[(b, jc)][0:jcs, h, :],
                                            ets[jc][0:jcs, hi, :],
                                            start=(jc == 0), stop=(jc == 2))
                                rst = rs_p.tile([64, 2, S], F32,
                                                name="rst", tag="rst")
                                nc.vector.reciprocal_approx_fast(
                                    out=rst[0:64, :, :],
                                    in_=cp[0:64, :, 0:S])
                                for hi in range(2):
                                    nc.vector.tensor_mul(
                                        out=ctxT[ec][hi * 64:hi * 64 + 64,
                                                     b * S:(b + 1) * S],
                                        in0=cp[64:128, hi, 0:S],
                                        in1=rst[0:64, hi, :])

                # residual x^T (f32) loads late - DMA is idle by now
                xt = {}
                for b in range(B_LOC):
                    for k in range(KC):
                        t = xt_p.tile([128, S], F32, name="xt", tag="xt")
                        nc.sync.dma_start(
                            out=t[:],
                            in_=xT_d[k * 128:(k + 1) * 128,
                                     b * S:(b + 1) * S])
                        xt[(k, b)] = t

                # xln1 closed; right-side long-lived pools
                ht_p = top.enter_context(
                    tc.tile_pool(name="ht", bufs=KC, side="right"))
                f1o_p = top.enter_context(
                    tc.tile_pool(name="f1o", bufs=MC_I, side="right"))

                # ============= out projection + residual =================
                ht = []
                with ExitStack() as ph:
                    wo_p = ph.enter_context(tc.tile_pool(name="wo", bufs=3))
                    ppo = ph.enter_context(
                        tc.tile_pool(name="ppo", bufs=2, space="PSUM"))
                    for m in range(MC_E):
                        wt = wo_p.tile([128, KC, 128], BF16, name="wo",
                                       tag="wo")
                        nc.sync.dma_start(out=wt[:], in_=ow_d[m, :, :, :])
                        ps = ppo.tile([128, B_LOC, 512], F32, name="po",
                                      tag="po")
                        for b in range(B_LOC):
                            for k in range(KC):
                                nc.tensor.matmul(
                                    ps[:, b, 0:S], wt[:, k, :],
                                    ctxT[k][:, b * S:(b + 1) * S],
                                    start=(k == 0), stop=(k == KC - 1))
                        o = ht_p.tile([128, NT], F32, name="ht", tag="ht")
                        for b in range(B_LOC):
                            nc.vector.scalar_tensor_tensor(
                                out=o[:, b * S:(b + 1) * S],
                                in0=ps[:, b, 0:S],
                                scalar=ob_sb[:, m:m + 1],
                                in1=xt[(m, b)][:],
                                op0=ALU.add, op1=ALU.add)
                        ht.append(o)
            # ctxT closed
        # xt closed

        # ================= LN2 + MLP =====================================
        with tc.tile_pool(name="xln2", bufs=KC * B_LOC) as xln2_p:
            f1o = []
            with ExitStack() as ln2_ph:
                htb_p = ln2_ph.enter_context(tc.tile_pool(name="htb", bufs=KC))
                htb = []
                for k in range(KC):
                    hb = htb_p.tile([128, NT], BF16, name="htb", tag="htb")
                    nc.vector.tensor_copy(out=hb[:], in_=ht[k][:])
                    htb.append(hb)
                xln2 = emit_ln(
                    ln2_ph,
                    lambda k, b: ht[k][:, b * S:(b + 1) * S],
                    lambda k, b: htb[k][:, b * S:(b + 1) * S],
                    xln2_p, "2")
                wf1_p = ln2_ph.enter_context(tc.tile_pool(name="wf1", bufs=3))
                ppf1 = ln2_ph.enter_context(
                    tc.tile_pool(name="ppf1", bufs=2, space="PSUM"))
                for m in range(MC_I):
                    wt = wf1_p.tile([128, KC, 128], BF16, name="wf1",
                                    tag="wf1")
                    nc.sync.dma_start(out=wt[:], in_=f1w_d[m, :, :, :])
                    o = f1o_p.tile([128, NT], BF16, name="f1o", tag="f1o")
                    for half in range(2):
                        ps = ppf1.tile([128, 2, 512], F32, name="pf1",
                                       tag="pf1")
                        for bb in range(2):
                            b = half * 2 + bb
                            for k in range(KC):
                                nc.tensor.matmul(
                                    ps[:, bb, 0:S], wt[:, k, :],
                                    xln2[(k, b)][:],
                                    start=(k == 0), stop=(k == KC - 1))
                        nc.scalar.activation(
                            out=o[:, half * 2 * S:(half + 1) * 2 * S],
                            in_=ps[:, :, 0:S],
                            func=AF.Gelu_apprx_tanh,
                            bias=f1b_sb[:, m:m + 1])
                    f1o.append(o)

        with ExitStack() as ph:
            wf2_p = ph.enter_context(tc.tile_pool(name="wf2", bufs=2))
            ppf2 = ph.enter_context(
                tc.tile_pool(name="ppf2", bufs=2, space="PSUM"))
            out_p = ph.enter_context(tc.tile_pool(name="outp", bufs=3))
            for m in range(MC_E):
                wt = wf2_p.tile([128, MC_I, 128], BF16, name="wf2", tag="wf2")
                nc.sync.dma_start(out=wt[:], in_=f2w_d[m, :, :, :])
                ps = ppf2.tile([128, B_LOC, 512], F32, name="pf2", tag="pf2")
                for b in range(B_LOC):
                    for k in range(MC_I):
                        nc.tensor.matmul(
                            ps[:, b, 0:S], wt[:, k, :],
                            f1o[k][:, b * S:(b + 1) * S],
                            start=(k == 0), stop=(k == MC_I - 1))
                o = out_p.tile([128, NT], F32, name="oo", tag="oo")
                nc.vector.scalar_tensor_tensor(
                    out=o[:], in0=ps[:, :, 0:S], scalar=f2b_sb[:, m:m + 1],
                    in1=ht[m][:], op0=ALU.add, op1=ALU.add)
                nc.sync.dma_start(out=outT_d[m * 128:(m + 1) * 128, :],
                                  in_=o[:])

    nc.compile()
    return nc


def _pack_lhsT(W):
    """W [M, K] (out, in) -> [M/128, 128, K/128, 128] bf16 with
    [m, p, k, j] = W[m*128+j, k*128+p] (lhsT tiles, partition = K)."""
    W = np.asarray(W, np.float32)
    M, K = W.shape
    A = W.reshape(M // 128, 128, K // 128, 128)
    return np.ascontiguousarray(A.transpose(0, 3, 2, 1)).astype(ml_dtypes.bfloat16)


def _pack_pbias(b):
    """b [M] -> [128, M/128] f32 per-partition bias columns."""
    return np.ascontiguousarray(np.asarray(b, np.float32).reshape(-1, 128).T)


def kernel(hidden_states, attention_mask, causal_attention_mask,
           ln1_w, ln1_b, q_w, q_b, k_w, k_b, v_w, v_b, o_w, o_b,
           ln2_w, ln2_b, fc1_w, fc1_b, fc2_w, fc2_b):
    global LAST_EXEC_NS
    from concourse.bass_utils import run_bass_kernel_spmd

    hs = np.asarray(hidden_states, np.float32)
    msk = (np.asarray(attention_mask, np.float32)
           + np.asarray(causal_attention_mask, np.float32))
    with_mask = bool(np.any(msk))

    ln1_w = np.asarray(ln1_w, np.float32); ln1_b = np.asarray(ln1_b, np.float32)
    ln2_w = np.asarray(ln2_w, np.float32); ln2_b = np.asarray(ln2_b, np.float32)
    q_w = np.asarray(q_w, np.float32); q_b = np.asarray(q_b, np.float32)
    k_w = np.asarray(k_w, np.float32); k_b = np.asarray(k_b, np.float32)
    v_w = np.asarray(v_w, np.float32); v_b = np.asarray(v_b, np.float32)
    o_w = np.asarray(o_w, np.float32); o_b = np.asarray(o_b, np.float32)
    fc1_w = np.asarray(fc1_w, np.float32); fc1_b = np.asarray(fc1_b, np.float32)
    fc2_w = np.asarray(fc2_w, np.float32); fc2_b = np.asarray(fc2_b, np.float32)

    scale = D ** -0.5
    # fold LN1 scale/bias into Q/K/V, and the softmax scale into Q
    qw_eff = (q_w * ln1_w[None, :]) * scale
    qb_eff = (q_b + q_w @ ln1_b) * scale
    kw_eff = k_w * ln1_w[None, :]
    kb_eff = k_b + k_w @ ln1_b
    vw_eff = v_w * ln1_w[None, :]
    vb_eff = v_b + v_w @ ln1_b
    # fold LN2 into fc1
    f1w_eff = fc1_w * ln2_w[None, :]
    f1b_eff = fc1_b + fc1_w @ ln2_b

    base = {
        "qw": _pack_lhsT(qw_eff),
        "kw": _pack_lhsT(kw_eff),
        "vw": np.ascontiguousarray(
            vw_eff.T.reshape(KC, 128, E)).astype(ml_dtypes.bfloat16),
        "ow": _pack_lhsT(o_w),
        "f1w": _pack_lhsT(f1w_eff),
        "f2w": _pack_lhsT(fc2_w),
        "qb": _pack_pbias(qb_eff),
        "kb": _pack_pbias(kb_eff),
        "vb": np.ascontiguousarray(vb_eff[None, :].astype(np.float32)),
        "ob": _pack_pbias(o_b),
        "f1b": _pack_pbias(f1b_eff),
        "f2b": _pack_pbias(fc2_b),
    }

    with_vbias = bool(np.any(vb_eff))
    with_qkbias = bool(np.any(qb_eff)) or bool(np.any(kb_eff))
    key = (with_mask, with_vbias, with_qkbias)
    if key not in _cache:
        _cache[key] = _build(with_mask, with_vbias, with_qkbias)
    nc = _cache[key]

    in_maps = []
    for c in range(N_CORES):
        x = hs[c * B_LOC:(c + 1) * B_LOC].reshape(NT, E).T
        m = dict(base)
        m["xT"] = np.ascontiguousarray(x)
        m["xTb"] = np.ascontiguousarray(x).astype(ml_dtypes.bfloat16)
        if with_mask:
            m["mskT"] = np.ascontiguousarray(
                msk[c * B_LOC:(c + 1) * B_LOC, 0].transpose(0, 2, 1))
        in_maps.append(m)

    res = run_bass_kernel_spmd(nc, in_maps, core_ids=list(range(N_CORES)),
                               trace=TRACE)
    LAST_EXEC_NS = res.exec_time_ns

    outs = []
    for c in range(N_CORES):
        oT = res.results[c]["outT"]          # [E, NT] f32
        outs.append(np.ascontiguousarray(oT.T).reshape(B_LOC, S, E))
    return np.concatenate(outs, axis=0)
